# revision 1
# baseline (speedup 1.0000x reference)
"""GCN node classifier (2x spmm + classifier + log_softmax) on 8 trn2 cores.

Strategy: destination-node 1D sharding. Each core owns 12,500 dst nodes and
the edges pointing at them. Layer tables (x@W1+b1, relu(h1)@W2+b2) are
node-major bf16 rows in DRAM; per-edge source rows are fetched with GPSIMD
dma_gather (int16 indices, so the table is addressed in 4 quarter views).
The segment-sum is a tensor-engine matmul against per-chunk scatter matrices
V[e, dst_lane] = edge_val[e] built on DVE with (iota == ldst) * val.
Between layers the per-shard T2 table is AllGather'ed into a Shared DRAM
tensor. All accumulation is f32 (PSUM); only table values are bf16.
"""

import numpy as np
import ml_dtypes

from contextlib import ExitStack


# ---------------------------------------------------------------- config ---
class Cfg:
    M = 8                 # cores
    N_NODES = 100000
    N_EDGES = 1600000
    IN_DIM = 128
    HID = 64
    NCLS = 40
    SHARD = 12500         # real dst nodes per core
    NT = 98               # dst tiles per core (128 each)
    KSEG = 5              # chunks (of 128 edges) per (tile, quarter) segment
    SLABC = 49            # chunks per gather slab
    X_BF16 = True         # phase-A (x@W1) in bf16
    SINGLE_PACKET = False  # multi-packet gathers (single-packet hangs >~1K idxs)
    NQUEUES = 4           # spread gathers over all 4 SWDGE queues

    @property
    def PADSHARD(self):
        return self.NT * 128

    @property
    def NPAD(self):
        return self.PADSHARD * self.M

    @property
    def QROWS(self):
        return self.NPAD // 4

    @property
    def SEG(self):
        return self.KSEG * 128

    @property
    def CQ(self):
        return self.NT * self.KSEG          # chunks per quarter

    @property
    def NSLAB(self):
        assert self.CQ % self.SLABC == 0
        return self.CQ // self.SLABC        # gather slabs per quarter

    @property
    def CHUNKS(self):
        return 4 * self.CQ

    @property
    def ASLAB(self):
        # phase-A node slab: 2048 nodes (16 x 128)
        assert self.NPAD % 2048 == 0
        return self.NPAD // 2048


CFG = Cfg()


# ------------------------------------------------------------- host plan ---
def _plan(cfg, edge_row, edge_col, edge_val):
    """Bucket/sort/pad edges per core. Returns per-core arrays:
    idx16 [128, 4*CQ*128/16] int16, ldstT [128, CHUNKS] f32, valT [128, CHUNKS] f32.
    """
    M, SHARD, PADSHARD = cfg.M, cfg.SHARD, cfg.PADSHARD
    NT, KSEG, SEG, CQ, QROWS = cfg.NT, cfg.KSEG, cfg.SEG, cfg.CQ, cfg.QROWS

    # padded (table) node id and quarter decomposition of sources
    psrc = (edge_col // SHARD) * PADSHARD + (edge_col % SHARD)
    q_of = psrc // QROWS
    i_of = psrc % QROWS
    core_of = edge_row // SHARD
    dloc = edge_row % SHARD
    t_of = dloc // 128
    l_of = dloc % 128

    L = 4 * CQ * 128
    idx_all, ldst_all, val_all = [], [], []
    for c in range(M):
        sel = core_of == c
        # order: (quarter, tile) segment id
        segid = q_of[sel] * NT + t_of[sel]
        order = np.argsort(segid, kind="stable")
        sid = segid[order]
        idx_s = i_of[sel][order]
        l_s = l_of[sel][order]
        v_s = edge_val[sel][order]

        counts = np.bincount(sid, minlength=4 * NT)
        if counts.max() > SEG:
            raise ValueError(f"segment overflow: {counts.max()} > {SEG}")
        # place into padded stream: segment s at offset s*SEG
        starts = np.arange(4 * NT) * SEG
        pos = starts[sid] + (np.arange(sid.size) - np.concatenate(([0], np.cumsum(counts)))[sid])

        idx = np.zeros(L, dtype=np.int16)
        ldst = np.zeros(L, dtype=np.float32)
        val = np.zeros(L, dtype=np.float32)
        idx[pos] = idx_s.astype(np.int16)
        ldst[pos] = l_s.astype(np.float32)
        val[pos] = v_s.astype(np.float32)

        # wrap indices: idx i -> [i%16, i//16], replicated on all 8 q7 cores
        idxw = np.tile(idx.reshape(-1, 16).T, (8, 1)).copy()          # [128, L/16]
        ldstT = np.ascontiguousarray(ldst.reshape(-1, 128).T)         # [128, CHUNKS]
        valT = np.ascontiguousarray(val.reshape(-1, 128).T)
        idx_all.append(idxw)
        ldst_all.append(ldstT)
        val_all.append(valT)
    return idx_all, ldst_all, val_all


def _pack_x(cfg, x):
    """x [N, IN] -> padded transposed [IN, NPAD] (f32 or bf16)."""
    xp = np.zeros((cfg.NPAD, cfg.IN_DIM), dtype=np.float32)
    xp.reshape(cfg.M, cfg.PADSHARD, cfg.IN_DIM)[:, : cfg.SHARD] = x.reshape(
        cfg.M, cfg.SHARD, cfg.IN_DIM
    )
    xT = np.ascontiguousarray(xp.T)
    if cfg.X_BF16:
        xT = xT.astype(ml_dtypes.bfloat16)
    return xT


# --------------------------------------------------------- device program ---
def _build(cfg, timing=False):
    from concourse import bacc, tile
    import concourse.mybir as mybir

    f32 = mybir.dt.float32
    bf16 = mybir.dt.bfloat16
    i16 = mybir.dt.int16
    AOP = mybir.AluOpType
    ACT = mybir.ActivationFunctionType

    xdt = bf16 if cfg.X_BF16 else f32

    nc = bacc.Bacc("TRN2", target_bir_lowering=False, debug=False,
                   num_devices=1 if timing else cfg.M,
                   dynamic_dma_scratch_size=getattr(cfg, "DMA_SCRATCH", 16384),
                   num_swdge_queues=getattr(cfg, "NQUEUES", 1))

    NPAD, QROWS, NT, KSEG, CQ, SLABC, NSLAB = (
        cfg.NPAD, cfg.QROWS, cfg.NT, cfg.KSEG, cfg.CQ, cfg.SLABC, cfg.NSLAB)
    CHUNKS, HID, NCLS, IN_DIM = cfg.CHUNKS, cfg.HID, cfg.NCLS, cfg.IN_DIM
    LQ16 = CQ * 128 // 16              # idx columns per quarter
    SLAB16 = SLABC * 128 // 16         # idx columns per slab
    NA = cfg.ASLAB                     # phase-A slabs (2048 nodes each)

    # -------- I/O
    XT = nc.dram_tensor("xt", [IN_DIM, NPAD], xdt, kind="ExternalInput")
    IDX = nc.dram_tensor("idx", [128, 4 * LQ16], i16, kind="ExternalInput")
    LDST = nc.dram_tensor("ldst", [128, CHUNKS], f32, kind="ExternalInput")
    VAL = nc.dram_tensor("val", [128, CHUNKS], f32, kind="ExternalInput")
    W1 = nc.dram_tensor("w1", [IN_DIM, HID], xdt, kind="ExternalInput")
    W2 = nc.dram_tensor("w2", [HID, HID], f32, kind="ExternalInput")
    WC = nc.dram_tensor("wc", [HID, NCLS], f32, kind="ExternalInput")
    B1 = nc.dram_tensor("b1", [128, HID], f32, kind="ExternalInput")   # replicated
    B2 = nc.dram_tensor("b2", [128, HID], f32, kind="ExternalInput")
    BC = nc.dram_tensor("bc", [128, NCLS], f32, kind="ExternalInput")
    IOTA = nc.dram_tensor("iota", [128, 128], bf16, kind="ExternalInput")
    IDENT = nc.dram_tensor("ident", [128, 128], f32, kind="ExternalInput")
    OUT = nc.dram_tensor("out", [cfg.PADSHARD, NCLS], f32, kind="ExternalOutput")

    # -------- internal DRAM
    T1 = nc.dram_tensor("t1tab", [NPAD, 128], bf16)                 # cols 64: junk
    T2S = nc.dram_tensor("t2shard", [cfg.PADSHARD, 128], bf16)
    T2F = nc.dram_tensor("t2full", [NPAD, 128], bf16, addr_space="Shared")

    with tile.TileContext(nc) as tc, ExitStack() as top:
        cpool = top.enter_context(tc.tile_pool(name="consts", bufs=1))
        w1s = cpool.tile([IN_DIM, HID], xdt)
        nc.sync.dma_start(out=w1s, in_=W1[:, :])
        w2s = cpool.tile([HID, HID], f32)
        nc.sync.dma_start(out=w2s, in_=W2[:, :])
        wcs = cpool.tile([HID, NCLS], f32)
        nc.sync.dma_start(out=wcs, in_=WC[:, :])
        b1s = cpool.tile([128, HID], f32)
        nc.sync.dma_start(out=b1s, in_=B1[:, :])
        b2s = cpool.tile([128, HID], f32)
        nc.sync.dma_start(out=b2s, in_=B2[:, :])
        bcs = cpool.tile([128, NCLS], f32)
        nc.sync.dma_start(out=bcs, in_=BC[:, :])
        b18s = cpool.tile([128, 8, HID], f32)
        for r in range(8):
            nc.sync.dma_start(out=b18s[:, r, :], in_=B1[:, :])
        iot = cpool.tile([128, 128], bf16)
        nc.sync.dma_start(out=iot, in_=IOTA[:, :])
        idn = cpool.tile([128, 128], f32)
        nc.sync.dma_start(out=idn, in_=IDENT[:, :])

        edg = top.enter_context(tc.tile_pool(name="edg", bufs=1))
        ldsts = edg.tile([128, CHUNKS], f32)
        nc.sync.dma_start(out=ldsts, in_=LDST[:, :])
        vals = edg.tile([128, CHUNKS], f32)
        nc.sync.dma_start(out=vals, in_=VAL[:, :])

        accp = top.enter_context(tc.tile_pool(name="acc", bufs=1))

        # ================= phase A: T1 = x @ W1 + b1 (node-major bf16 rows)
        with tc.tile_pool(name="xa", bufs=3) as xa, \
             tc.tile_pool(name="sta", bufs=3) as sta, \
             tc.tile_pool(name="psa", bufs=4, space="PSUM") as psa:
            for s in range(NA):
                xs = xa.tile([128, 2048], xdt)
                nc.sync.dma_start(out=xs, in_=XT[:, s * 2048:(s + 1) * 2048])
                st = sta.tile([128, 16, HID], bf16)
                for h in range(2):
                    pb = psa.tile([128, 8, HID], f32)
                    for k8 in range(8):
                        k = h * 8 + k8
                        nc.tensor.matmul(pb[:, k8, :],
                                         lhsT=xs[:, k * 128:(k + 1) * 128],
                                         rhs=w1s, start=True, stop=True)
                    nc.vector.tensor_tensor(st[:, h * 8:(h + 1) * 8, :], pb,
                                            b18s, AOP.add)
                dst = T1[s * 2048:(s + 1) * 2048, 0:HID].rearrange(
                    "(k p) f -> p k f", p=128)
                nc.sync.dma_start(out=dst, in_=st)

        # ============ spmm layer runner: per-tile single psum group across
        # all 4 quarters (slabs for all quarters retire in lockstep), with a
        # fused per-tile epilogue.
        def spmm_layer(tab, epilogue, pools):
            idxp, msg, vp, psb = pools
            its = []
            slabs = [[None] * NSLAB for _ in range(4)]
            for q in range(4):
                it = idxp.tile([128, LQ16], i16, tag=f"idx{q}")
                nc.sync.dma_start(out=it, in_=IDX[:, q * LQ16:(q + 1) * LQ16])
                its.append(it)

            def ensure_slab(q, s):
                if slabs[q][s] is None:
                    mt = msg.tile([128, SLABC, 128], bf16)
                    nc.gpsimd.dma_gather(
                        mt, tab[q * QROWS:(q + 1) * QROWS, :],
                        its[q][:, s * SLAB16:(s + 1) * SLAB16],
                        num_idxs=SLABC * 128, num_idxs_reg=SLABC * 128,
                        elem_size=128, elem_step=128,
                        single_packet=getattr(cfg, "SINGLE_PACKET", True),
                        queue_num=(q * NSLAB + s) % getattr(cfg, "NQUEUES", 1))
                    slabs[q][s] = mt
                return slabs[q][s]

            for t in range(NT):
                ps = psb.tile([128, HID], f32)
                for q in range(4):
                    for k in range(KSEG):
                        j = t * KSEG + k                 # chunk in quarter
                        gj = q * CQ + j                  # global chunk
                        v = vp.tile([128, 128], bf16)
                        nc.vector.tensor_scalar(
                            v, iot, ldsts[:, gj:gj + 1], vals[:, gj:gj + 1],
                            AOP.is_equal, AOP.mult)
                        mt = ensure_slab(q, j // SLABC)
                        nc.tensor.matmul(ps, lhsT=v,
                                         rhs=mt[:, j % SLABC, 0:HID],
                                         start=(q == 0 and k == 0),
                                         stop=(q == 3 and k == KSEG - 1))
                epilogue(t, ps)

        # ================= layer 1 + phase C fused: T2S = relu(h1)@W2+b2
        for _rep in range(getattr(cfg, "REPS", 1)):
            _run_layers(cfg, nc, tc, tile, mybir, timing, accp, locals())
    nc.compile()
    return nc


def _run_layers(cfg, nc, tc, tile, mybir, timing, accp, env):
    f32 = mybir.dt.float32
    bf16 = mybir.dt.bfloat16
    i16 = mybir.dt.int16
    AOP = mybir.AluOpType
    ACT = mybir.ActivationFunctionType
    NPAD, QROWS, NT, KSEG, CQ, SLABC, NSLAB = (
        cfg.NPAD, cfg.QROWS, cfg.NT, cfg.KSEG, cfg.CQ, cfg.SLABC, cfg.NSLAB)
    CHUNKS, HID, NCLS, IN_DIM = cfg.CHUNKS, cfg.HID, cfg.NCLS, cfg.IN_DIM
    LQ16 = CQ * 128 // 16
    SLAB16 = SLABC * 128 // 16
    (T1, T2S, T2F, IDX, OUT, iot, idn, ldsts, vals, w2s, wcs, b2s, bcs) = (
        env["T1"], env["T2S"], env["T2F"], env["IDX"], env["OUT"], env["iot"],
        env["idn"], env["ldsts"], env["vals"], env["w2s"], env["wcs"],
        env["b2s"], env["bcs"])
    spmm_layer = env["spmm_layer"]

    if True:
        with tc.tile_pool(name="idxp", bufs=getattr(cfg, "IDXBUFS", 2)) as idxp, \
             tc.tile_pool(name="msg", bufs=getattr(cfg, "MSGBUFS", 8)) as msg, \
             tc.tile_pool(name="vp", bufs=8) as vp, \
             tc.tile_pool(name="psb", bufs=3, space="PSUM") as psb, \
             tc.tile_pool(name="tc1", bufs=3) as tp1, \
             tc.tile_pool(name="tc2", bufs=3) as tp2, \
             tc.tile_pool(name="tc3", bufs=3) as tp3, \
             tc.tile_pool(name="pst", bufs=2, space="PSUM") as pst, \
             tc.tile_pool(name="psc", bufs=2, space="PSUM") as psc:

            def epi1(t, ps):
                h1r = tp1.tile([128, HID], f32)
                nc.scalar.activation(h1r, ps, ACT.Relu)
                ptr = pst.tile([HID, 128], f32)
                nc.tensor.transpose(ptr, h1r, idn)
                h1t = tp2.tile([HID, 128], f32)
                nc.vector.tensor_copy(out=h1t, in_=ptr)
                ps2 = psc.tile([128, HID], f32)
                nc.tensor.matmul(ps2, lhsT=h1t, rhs=w2s, start=True, stop=True)
                t2t = tp3.tile([128, HID], bf16)
                nc.vector.tensor_tensor(t2t, ps2, b2s, AOP.add)
                nc.sync.dma_start(out=T2S[t * 128:(t + 1) * 128, 0:HID], in_=t2t)

            spmm_layer(T1, epi1, (idxp, msg, vp, psb))
            if not timing:
                nc.gpsimd.collective_compute(
                    "AllGather", mybir.AluOpType.bypass,
                    replica_groups=[list(range(cfg.M))],
                    ins=[T2S[:, :]], outs=[T2F[:, :]])

        # ================= layer 2 + phase E fused: logits + log_softmax
        with tc.tile_pool(name="idxp2", bufs=getattr(cfg, "IDXBUFS", 2)) as idxp2, \
             tc.tile_pool(name="msg2", bufs=getattr(cfg, "MSGBUFS", 8)) as msg2, \
             tc.tile_pool(name="vp2", bufs=8) as vp2, \
             tc.tile_pool(name="psb2", bufs=3, space="PSUM") as psb2, \
             tc.tile_pool(name="te1", bufs=3) as te1, \
             tc.tile_pool(name="te2", bufs=3) as te2, \
             tc.tile_pool(name="pse", bufs=2, space="PSUM") as pse, \
             tc.tile_pool(name="psf", bufs=2, space="PSUM") as psf:
            lgacc = accp.tile([128, NT, NCLS], f32, tag="lgacc")
            negmacc = accp.tile([128, NT], f32, tag="negmacc")
            smacc = accp.tile([128, NT], f32, tag="smacc")

            def epi2(t, ps):
                h2s = te1.tile([128, HID], f32)
                nc.scalar.activation(h2s, ps, ACT.Copy)
                ptr = pse.tile([HID, 128], f32)
                nc.tensor.transpose(ptr, h2s, idn)
                h2t = te2.tile([HID, 128], f32)
                nc.vector.tensor_copy(out=h2t, in_=ptr)
                psl = psf.tile([128, NCLS], f32)
                nc.tensor.matmul(psl, lhsT=h2t, rhs=wcs, start=True, stop=True)
                nc.vector.tensor_tensor(lgacc[:, t, :], psl, bcs, AOP.add)
                nc.vector.tensor_reduce(negmacc[:, t:t + 1], lgacc[:, t, :],
                                        mybir.AxisListType.X, AOP.max,
                                        negate=True)
                et = te1.tile([128, NCLS], f32, tag="et")
                nc.scalar.activation(et, lgacc[:, t, :], ACT.Exp,
                                     bias=negmacc[:, t:t + 1],
                                     accum_out=smacc[:, t:t + 1])

            spmm_layer(T2F, epi2, (idxp2, msg2, vp2, psb2))

            # one Ln over all tiles, then final subtract + store
            lnacc = accp.tile([128, NT], f32, tag="lnacc")
            nc.scalar.activation(lnacc, smacc, ACT.Ln)
            shacc = accp.tile([128, NT], f32, tag="shacc")
            nc.vector.tensor_tensor(shacc, lnacc, negmacc, AOP.subtract)
            for t in range(NT):
                ot = te2.tile([128, NCLS], f32, tag="ot")
                nc.vector.tensor_scalar(ot, lgacc[:, t, :],
                                        shacc[:, t:t + 1], None, AOP.subtract)
                nc.sync.dma_start(out=OUT[t * 128:(t + 1) * 128, :], in_=ot)

    nc.compile()
    return nc


_NC_CACHE = {}


def _get_nc(cfg):
    key = (cfg.KSEG, cfg.X_BF16, getattr(cfg, "REPS", 1), cfg.SLABC)
    if key not in _NC_CACHE:
        _NC_CACHE[key] = _build(cfg)
    return _NC_CACHE[key]


# ------------------------------------------------------------------ main ---
def kernel(x, edge_row, edge_col, edge_val, W1, b1, W2, b2, Wc, bc,
           _run_kwargs=None):
    from concourse.bass_utils import run_bass_kernel_spmd

    cfg = CFG
    x = np.asarray(x, dtype=np.float32)
    edge_row = np.asarray(edge_row, dtype=np.int64)
    edge_col = np.asarray(edge_col, dtype=np.int64)
    edge_val = np.asarray(edge_val, dtype=np.float32)
    W1 = np.asarray(W1, dtype=np.float32)
    W2 = np.asarray(W2, dtype=np.float32)
    Wc = np.asarray(Wc, dtype=np.float32)
    b1 = np.asarray(b1, dtype=np.float32)
    b2 = np.asarray(b2, dtype=np.float32)
    bc = np.asarray(bc, dtype=np.float32)

    try:
        idx_all, ldst_all, val_all = _plan(cfg, edge_row, edge_col, edge_val)
    except ValueError:
        cfg.KSEG += 1
        idx_all, ldst_all, val_all = _plan(cfg, edge_row, edge_col, edge_val)

    xT = _pack_x(cfg, x)
    w1h = W1.astype(ml_dtypes.bfloat16) if cfg.X_BF16 else W1
    iota = np.tile(np.arange(128, dtype=np.float32), (128, 1)).astype(
        ml_dtypes.bfloat16)
    ident = np.eye(128, dtype=np.float32)
    b1r = np.tile(b1, (128, 1)).astype(np.float32)
    b2r = np.tile(b2, (128, 1)).astype(np.float32)
    bcr = np.tile(bc, (128, 1)).astype(np.float32)

    nc = _get_nc(cfg)
    in_maps = []
    for c in range(cfg.M):
        in_maps.append({
            "xt": xT, "idx": idx_all[c], "ldst": ldst_all[c],
            "val": val_all[c], "w1": w1h, "w2": W2, "wc": Wc,
            "b1": b1r, "b2": b2r, "bc": bcr, "iota": iota, "ident": ident,
        })
    kw = dict(_run_kwargs or {})
    res = run_bass_kernel_spmd(nc, in_maps, core_ids=list(range(cfg.M)), **kw)
    out = np.concatenate(
        [res.results[c]["out"][: cfg.SHARD] for c in range(cfg.M)], axis=0)
    kernel.last_results = res
    return out.astype(np.float32)



# revision 16
# speedup vs baseline: 1.4124x; 1.4124x over previous
"""GCN node classifier (2x spmm + classifier + log_softmax) on 8 trn2 cores.

Strategy: destination-node 1D sharding with spmm linearity.
  spmm(A, x@W1 + b1) = (A x)@W1 + deg * b1^T      (deg = rowsum of A)
  spmm(A, h@W2 + b2)@Wc = (A h)@(W2 Wc) + deg * (b2 Wc)^T
so the gather tables are the RAW node features (x bf16 for layer 1,
relu-h bf16 for layer 2) — no dense pre-pass over all nodes, and the
layer weights are applied per dst tile after aggregation.

Each core owns 12,800 dst slots (100 tiles x 128 lanes). Host assigns
nodes to slots with a greedy 4-d balancer so that every (src-quarter,
dst-tile) edge bucket fits in KSEG=4 chunks of 128 edges (the int16
gather index forces 4 quarter views of the 102,400-row table). Per-edge
source rows are fetched with GPSIMD dma_gather (256B rows); the
segment-sum is a tensor-engine matmul against per-chunk scatter
matrices V[e, dst_lane] = edge_val[e] built on DVE with
(iota == ldst) * val, accumulated transposed (psT = Xg^T V) so the
per-tile epilogue can feed psT straight back as lhsT for the weight
matmul. log-softmax is fused per tile. Between layers the per-shard
relu-h table is AllGather'ed into a Shared DRAM tensor.
"""

import numpy as np
import ml_dtypes

from contextlib import ExitStack


# ---------------------------------------------------------------- config ---
class Cfg:
    M = 8                 # cores
    N_NODES = 100000
    N_EDGES = 1600000
    IN_DIM = 128
    HID = 64
    NCLS = 40
    NT = 100              # dst tiles per core (128 lanes each)
    KSEG = 4              # chunks (of 128 edges) per (quarter, tile) segment
    SLABC = 50            # chunks per gather slab
    SINGLE_PACKET = False  # multi-packet gathers (single-packet hangs >~1K idxs)
    NQUEUES = 4           # spread gathers over all 4 SWDGE queues
    MSGBUFS = 9
    IDXBUFS = 1
    GE = 10               # tiles per epilogue-matmul batch
    LNG = 20              # tiles per deferred-Ln group
    NZBIAS = False        # set per-input: any of b1/b2/bc nonzero

    @property
    def PADSHARD(self):
        return self.NT * 128

    @property
    def NPAD(self):
        return self.PADSHARD * self.M

    @property
    def QROWS(self):
        return self.NPAD // 4

    @property
    def SEG(self):
        return self.KSEG * 128

    @property
    def CQ(self):
        return self.NT * self.KSEG          # chunks per quarter

    @property
    def NSLAB(self):
        assert self.CQ % self.SLABC == 0
        return self.CQ // self.SLABC        # gather slabs per quarter

    @property
    def CHUNKS(self):
        return 4 * self.CQ


CFG = Cfg()


# ------------------------------------------------------------- host plan ---
def _assign_slots(cfg, edge_row, edge_col):
    """Assign nodes to table slots so every (src-quarter, dst-tile) edge
    bucket holds <= KSEG*128 edges. Returns slot_of[node] -> [0, NPAD).

    Nodes are first split into 4 fixed quarter groups (so each node's
    src-quarter is pinned), then greedily packed into the 2*NT tiles of
    their own quarter balancing the 4-vector of per-src-quarter in-edge
    counts.
    """
    N, NPAD, QROWS, NT, M = cfg.N_NODES, cfg.NPAD, cfg.QROWS, cfg.NT, cfg.M
    TPQ = QROWS // 128                       # tiles per quarter (2 cores)
    rng = np.random.default_rng(12345)
    order = rng.permutation(N)
    qgrp = np.empty(N, dtype=np.int64)       # node -> quarter group
    npq = N // 4
    for q in range(4):
        qgrp[order[q * npq:(q + 1) * npq]] = q
    qgrp[order[4 * npq:]] = 3

    # per-node in-edge count by source quarter
    cnt = np.zeros((N, 4), dtype=np.int64)
    np.add.at(cnt, (edge_row, qgrp[edge_col]), 1)

    slot_of = np.empty(N, dtype=np.int64)
    for q in range(4):
        nodes = np.where(qgrp == q)[0]
        c = cnt[nodes].astype(np.float32)            # [nq, 4]
        tot = c.sum(1)
        o = np.argsort(-tot, kind="stable")
        nodes, c = nodes[o], c[o]
        loads = np.zeros((TPQ, 4), dtype=np.float32)
        fill = np.zeros(TPQ, dtype=np.int64)
        pos = np.empty(nodes.size, dtype=np.int64)
        for i in range(nodes.size):
            cand = np.max(loads + c[i], axis=1) + (fill >= 128) * 1e9
            b = int(np.argmin(cand))
            loads[b] += c[i]
            pos[i] = b * 128 + fill[b]
            fill[b] += 1
        slot_of[nodes] = q * QROWS + pos
    return slot_of


def _plan(cfg, edge_row, edge_col, edge_val, slot_of):
    """Bucket/sort/pad edges per core. Returns per-core arrays:
    idx16 [128, 4*CQ*128/16] int16, ldstT/valT [128, CHUNKS] bf16,
    plus degs [128, NT] f32 per core.
    """
    M, NT, KSEG, SEG, CQ, QROWS = cfg.M, cfg.NT, cfg.KSEG, cfg.SEG, cfg.CQ, cfg.QROWS
    PADSHARD = cfg.PADSHARD

    src_slot = slot_of[edge_col]
    dst_slot = slot_of[edge_row]
    q_of = src_slot // QROWS
    i_of = src_slot % QROWS
    core_of = dst_slot // PADSHARD
    dloc = dst_slot % PADSHARD
    t_of = dloc // 128
    l_of = dloc % 128

    deg = np.zeros(cfg.NPAD, dtype=np.float64)
    np.add.at(deg, dst_slot, edge_val.astype(np.float64))

    L = 4 * CQ * 128
    idx_all, ldst_all, val_all, deg_all = [], [], [], []
    for c in range(M):
        sel = core_of == c
        segid = q_of[sel] * NT + t_of[sel]
        order = np.argsort(segid, kind="stable")
        sid = segid[order]
        idx_s = i_of[sel][order]
        l_s = l_of[sel][order]
        v_s = edge_val[sel][order]

        counts = np.bincount(sid, minlength=4 * NT)
        if counts.max() > SEG:
            raise ValueError(f"segment overflow: {counts.max()} > {SEG}")
        starts = np.arange(4 * NT) * SEG
        pos = starts[sid] + (np.arange(sid.size)
                             - np.concatenate(([0], np.cumsum(counts)))[sid])

        idx = np.zeros(L, dtype=np.int16)
        ldst = np.zeros(L, dtype=np.float32)
        val = np.zeros(L, dtype=np.float32)
        idx[pos] = idx_s.astype(np.int16)
        ldst[pos] = l_s.astype(np.float32)
        val[pos] = v_s.astype(np.float32)

        # wrap indices: idx i -> [i%16, i//16], replicated on all 8 q7 cores
        idxw = np.tile(idx.reshape(-1, 16).T, (8, 1)).copy()
        ldstT = np.ascontiguousarray(ldst.reshape(-1, 128).T)
        valT = np.ascontiguousarray(val.reshape(-1, 128).T)
        degs = np.ascontiguousarray(
            deg[c * PADSHARD:(c + 1) * PADSHARD].reshape(NT, 128).T
        ).astype(np.float32)
        idx_all.append(idxw)
        ldst_all.append(ldstT)
        val_all.append(valT)
        deg_all.append(degs)
    return idx_all, ldst_all, val_all, deg_all


# --------------------------------------------------------- device program ---
def _build(cfg, timing=False, nzbias=False):
    import os
    from concourse import bacc, tile
    import concourse.mybir as mybir
    kdbg = bool(os.environ.get("KDBG"))

    f32 = mybir.dt.float32
    bf16 = mybir.dt.bfloat16
    i16 = mybir.dt.int16
    AOP = mybir.AluOpType
    ACT = mybir.ActivationFunctionType

    nc = bacc.Bacc("TRN2", target_bir_lowering=False, debug=False,
                   num_devices=1 if timing else cfg.M,
                   dynamic_dma_scratch_size=16384,
                   num_swdge_queues=cfg.NQUEUES)

    NPAD, QROWS, NT, KSEG, CQ, SLABC, NSLAB = (
        cfg.NPAD, cfg.QROWS, cfg.NT, cfg.KSEG, cfg.CQ, cfg.SLABC, cfg.NSLAB)
    CHUNKS, HID, NCLS, IN_DIM = cfg.CHUNKS, cfg.HID, cfg.NCLS, cfg.IN_DIM
    LQ16 = CQ * 128 // 16              # idx columns per quarter
    SLAB16 = SLABC * 128 // 16         # idx columns per slab

    # -------- I/O
    XG = nc.dram_tensor("xg", [NPAD, IN_DIM], bf16, kind="ExternalInput")
    IDX = nc.dram_tensor("idx", [128, 4 * LQ16], i16, kind="ExternalInput")
    LDST = nc.dram_tensor("ldst", [128, CHUNKS], f32, kind="ExternalInput")
    VAL = nc.dram_tensor("val", [128, CHUNKS], f32, kind="ExternalInput")
    DEG = nc.dram_tensor("deg", [128, NT], f32, kind="ExternalInput")
    W1 = nc.dram_tensor("w1", [IN_DIM, HID], bf16, kind="ExternalInput")
    W2C = nc.dram_tensor("w2c", [HID, NCLS], bf16, kind="ExternalInput")
    B1R = nc.dram_tensor("b1r", [128, HID], f32, kind="ExternalInput")
    BCOMBR = nc.dram_tensor("bcombr", [128, NCLS], f32, kind="ExternalInput")
    BCR = nc.dram_tensor("bcr", [128, NCLS], f32, kind="ExternalInput")
    IOTA = nc.dram_tensor("iota", [128, 128], bf16, kind="ExternalInput")
    OUT = nc.dram_tensor("out", [cfg.PADSHARD, NCLS], f32, kind="ExternalOutput")
    HDBG = (nc.dram_tensor("hdbg", [cfg.PADSHARD, HID], bf16,
                           kind="ExternalOutput") if kdbg else None)
    LDBG = (nc.dram_tensor("ldbg", [128, NT * NCLS], f32,
                           kind="ExternalOutput") if kdbg else None)
    SMDBG = (nc.dram_tensor("smdbg", [128, NT], f32,
                            kind="ExternalOutput") if kdbg else None)

    # -------- internal DRAM
    HS = nc.dram_tensor("hshard", [cfg.PADSHARD, 128], bf16)    # cols 64+: junk
    HF = nc.dram_tensor("hfull", [NPAD, 128], bf16, addr_space="Shared")

    with tile.TileContext(nc) as tc, ExitStack() as top:
        cpool = top.enter_context(tc.tile_pool(name="consts", bufs=1))
        w1s = cpool.tile([IN_DIM, HID], bf16)
        nc.sync.dma_start(out=w1s, in_=W1[:, :])
        w2cs = cpool.tile([HID, NCLS], bf16)
        nc.sync.dma_start(out=w2cs, in_=W2C[:, :])
        b1rs = cpool.tile([128, HID], f32)
        nc.sync.dma_start(out=b1rs, in_=B1R[:, :])
        bcombs = cpool.tile([128, NCLS], f32)
        nc.sync.dma_start(out=bcombs, in_=BCOMBR[:, :])
        bcrs = cpool.tile([128, NCLS], f32)
        nc.sync.dma_start(out=bcrs, in_=BCR[:, :])
        iot = cpool.tile([128, 128], bf16)
        nc.sync.dma_start(out=iot, in_=IOTA[:, :])
        degs = cpool.tile([128, NT], f32)
        nc.sync.dma_start(out=degs, in_=DEG[:, :])

        edg = top.enter_context(tc.tile_pool(name="edg", bufs=1))
        ldsts = edg.tile([128, CHUNKS], f32)
        nc.sync.dma_start(out=ldsts, in_=LDST[:, :])
        vals = edg.tile([128, CHUNKS], f32)
        nc.sync.dma_start(out=vals, in_=VAL[:, :])

        # idx tiles for BOTH layers, loaded up front
        idxp = top.enter_context(tc.tile_pool(name="idxp", bufs=1))
        its = []
        for li in range(2):
            row = []
            for q in range(4):
                it = idxp.tile([128, LQ16], i16, tag=f"idx{li}_{q}",
                               name=f"idx{li}_{q}")
                nc.sync.dma_start(out=it, in_=IDX[:, q * LQ16:(q + 1) * LQ16])
                row.append(it)
            its.append(row)

        # ============ spmm layer runner: per-tile single psum group across
        # all 4 quarters, accumulating transposed (psT = Xg^T V); epilogue
        # split into a per-tile part (cast) and a batched per-GE-tiles part
        # (weight matmuls etc) to keep the PE stream free of cross-engine
        # round trips.
        def spmm_layer(tab, width, lits, epi_tile, epi_group, pools):
            msg, vp, psb = pools
            GEB = cfg.GE
            slabs = [[None] * NSLAB for _ in range(4)]

            def ensure_slab(q, s):
                if slabs[q][s] is None:
                    mt = msg.tile([128, SLABC, 128], bf16)
                    nc.gpsimd.dma_gather(
                        mt, tab[q * QROWS:(q + 1) * QROWS, :],
                        lits[q][:, s * SLAB16:(s + 1) * SLAB16],
                        num_idxs=SLABC * 128, num_idxs_reg=SLABC * 128,
                        elem_size=128, elem_step=128,
                        single_packet=cfg.SINGLE_PACKET,
                        queue_num=(q * NSLAB + s) % cfg.NQUEUES)
                    slabs[q][s] = mt
                return slabs[q][s]

            for t in range(NT):
                ps = psb.tile([width, 128], f32)
                for q in range(4):
                    j0 = t * KSEG
                    vt = vp.tile([128, KSEG, 128], bf16)
                    for k in range(KSEG):
                        gj = q * CQ + j0 + k             # global chunk
                        nc.vector.tensor_scalar(
                            vt[:, k, :], iot, ldsts[:, gj:gj + 1],
                            vals[:, gj:gj + 1], AOP.is_equal, AOP.mult)
                    for k in range(KSEG):
                        j = j0 + k                       # chunk in quarter
                        mt = ensure_slab(q, j // SLABC)
                        nc.tensor.matmul(ps, lhsT=mt[:, j % SLABC, 0:width],
                                         rhs=vt[:, k, :],
                                         start=(q == 0 and k == 0),
                                         stop=(q == 3 and k == KSEG - 1))
                epi_tile(t, ps)
                if t % GEB == GEB - 1:
                    epi_group(t - GEB + 1, GEB)

        # ================= layer 1: h = relu((A x)@W1 + deg*b1^T), store bf16
        with tc.tile_pool(name="msg", bufs=cfg.MSGBUFS) as msg, \
             tc.tile_pool(name="vp", bufs=8) as vp, \
             tc.tile_pool(name="psb", bufs=3, space="PSUM") as psb, \
             tc.tile_pool(name="tc1", bufs=cfg.GE + 2) as tp1, \
             tc.tile_pool(name="tc2", bufs=3) as tp2, \
             tc.tile_pool(name="pse", bufs=3, space="PSUM") as pse:
            pss1 = {}

            def epi1_tile(t, ps):
                pss = tp1.tile([IN_DIM, 128], bf16, tag="pss", name="pss")
                nc.scalar.activation(pss, ps, ACT.Copy)
                pss1[t] = pss

            def epi1_group(t0, n):
                for t in range(t0, t0 + n):
                    ph = pse.tile([128, HID], f32)
                    nc.tensor.matmul(ph, lhsT=pss1.pop(t), rhs=w1s,
                                     start=True, stop=True)
                    ht = tp2.tile([128, HID], bf16, tag="ht")
                    if nzbias:
                        tb = tp2.tile([128, HID], f32, tag="tb")
                        nc.vector.tensor_scalar(tb, b1rs, degs[:, t:t + 1],
                                                None, AOP.mult)
                        hsum = tp2.tile([128, HID], f32, tag="hsum")
                        nc.vector.tensor_tensor(hsum, ph, tb, AOP.add)
                        nc.scalar.activation(ht, hsum, ACT.Relu)
                    else:
                        nc.scalar.activation(ht, ph, ACT.Relu)
                    nc.sync.dma_start(out=HS[t * 128:(t + 1) * 128, 0:HID],
                                      in_=ht)

            spmm_layer(XG, IN_DIM, its[0], epi1_tile, epi1_group,
                       (msg, vp, psb))
            if kdbg:
                nc.sync.dma_start(out=HDBG[:, :], in_=HS[:, 0:HID])
            if not timing:
                nc.gpsimd.collective_compute(
                    "AllGather", mybir.AluOpType.bypass,
                    replica_groups=[list(range(cfg.M))],
                    ins=[HS[:, :]], outs=[HF[:, :]])

        # ================= layer 2 + fused classifier/log_softmax
        with tc.tile_pool(name="msg2", bufs=cfg.MSGBUFS) as msg2, \
             tc.tile_pool(name="vp2", bufs=8) as vp2, \
             tc.tile_pool(name="psb2", bufs=3, space="PSUM") as psb2, \
             tc.tile_pool(name="te1", bufs=cfg.GE + 2) as te1, \
             tc.tile_pool(name="te2", bufs=3) as te2, \
             tc.tile_pool(name="te3", bufs=2) as te3, \
             tc.tile_pool(name="psf", bufs=3, space="PSUM") as psf:
            G = cfg.LNG
            assert NT % G == 0 and G % cfg.GE == 0
            pss2 = {}
            state = {}

            def epi2_tile(t, ps):
                pss = te1.tile([HID, 128], bf16, tag="pss", name="pss")
                nc.scalar.activation(pss, ps, ACT.Copy)
                pss2[t] = pss

            def epi2_group(t0, n):
                for t in range(t0, t0 + n):
                    g, i = t // G, t % G
                    if i == 0:
                        state["lgg"] = te3.tile([128, G, NCLS], f32,
                                                tag="lgg", name="lgg")
                        state["negg"] = te3.tile([128, G], f32,
                                                 tag="negg", name="negg")
                        state["smg"] = te3.tile([128, G], f32,
                                                tag="smg", name="smg")
                    lgg, negg, smg = state["lgg"], state["negg"], state["smg"]
                    psl = psf.tile([128, NCLS], f32)
                    nc.tensor.matmul(psl, lhsT=pss2.pop(t), rhs=w2cs,
                                     start=True, stop=True)
                    if nzbias:
                        tb = te2.tile([128, NCLS], f32, tag="tb")
                        nc.vector.tensor_scalar(tb, bcombs, degs[:, t:t + 1],
                                                None, AOP.mult)
                        lg0 = te2.tile([128, NCLS], f32, tag="lg0")
                        nc.vector.tensor_tensor(lg0, psl, tb, AOP.add)
                        nc.gpsimd.tensor_tensor(lgg[:, i, :], lg0, bcrs,
                                                AOP.add)
                    else:
                        nc.scalar.activation(lgg[:, i, :], psl, ACT.Copy)
                    if i == G - 1:
                        if kdbg:
                            nc.sync.dma_start(
                                out=LDBG[:, g * G * NCLS:(g + 1) * G * NCLS],
                                in_=lgg.rearrange("p a b -> p (a b)"))
                            nc.sync.dma_start(out=SMDBG[:, g * G:(g + 1) * G],
                                              in_=smg)
                        nc.vector.tensor_reduce(negg[:, :], lgg,
                                                mybir.AxisListType.X, AOP.max,
                                                negate=True)
                        for ii in range(G):
                            et = te2.tile([128, NCLS], f32, tag="et")
                            nc.scalar.activation(et, lgg[:, ii, :], ACT.Exp,
                                                 bias=negg[:, ii:ii + 1],
                                                 accum_out=smg[:, ii:ii + 1])
                        lng = te2.tile([128, G], f32, tag="lng")
                        nc.scalar.activation(lng, smg, ACT.Ln)
                        shg = te2.tile([128, G], f32, tag="shg")
                        nc.vector.tensor_tensor(shg, negg, lng, AOP.subtract)
                        for ii in range(G):
                            tt = g * G + ii
                            ot = te2.tile([128, NCLS], f32, tag="ot")
                            nc.vector.tensor_scalar(ot, lgg[:, ii, :],
                                                    shg[:, ii:ii + 1], None,
                                                    AOP.add)
                            nc.sync.dma_start(
                                out=OUT[tt * 128:(tt + 1) * 128, :], in_=ot)

            spmm_layer(HF, HID, its[1], epi2_tile, epi2_group,
                       (msg2, vp2, psb2))

    nc.compile()
    return nc


_NC_CACHE = {}


def _get_nc(cfg):
    key = (cfg.NT, cfg.KSEG, cfg.SLABC, cfg.NZBIAS)
    if key not in _NC_CACHE:
        _NC_CACHE[key] = _build(cfg, nzbias=cfg.NZBIAS)
    return _NC_CACHE[key]


# ------------------------------------------------------------------ main ---
def kernel(x, edge_row, edge_col, edge_val, W1, b1, W2, b2, Wc, bc,
           _run_kwargs=None):
    from concourse.bass_utils import run_bass_kernel_spmd

    cfg = CFG
    x = np.asarray(x, dtype=np.float32)
    edge_row = np.asarray(edge_row, dtype=np.int64)
    edge_col = np.asarray(edge_col, dtype=np.int64)
    edge_val = np.asarray(edge_val, dtype=np.float32)
    W1 = np.asarray(W1, dtype=np.float32)
    W2 = np.asarray(W2, dtype=np.float32)
    Wc = np.asarray(Wc, dtype=np.float32)
    b1 = np.asarray(b1, dtype=np.float32)
    b2 = np.asarray(b2, dtype=np.float32)
    bc = np.asarray(bc, dtype=np.float32)

    cfg.NZBIAS = bool(np.any(b1) or np.any(b2) or np.any(bc))
    slot_of = _assign_slots(cfg, edge_row, edge_col)
    try:
        idx_all, ldst_all, val_all, deg_all = _plan(
            cfg, edge_row, edge_col, edge_val, slot_of)
    except ValueError:
        cfg.KSEG += 1
        idx_all, ldst_all, val_all, deg_all = _plan(
            cfg, edge_row, edge_col, edge_val, slot_of)

    xg = np.zeros((cfg.NPAD, cfg.IN_DIM), dtype=ml_dtypes.bfloat16)
    xg[slot_of] = x.astype(ml_dtypes.bfloat16)

    w1h = W1.astype(ml_dtypes.bfloat16)
    w2c = (W2 @ Wc).astype(ml_dtypes.bfloat16)
    bcomb = b2 @ Wc
    iota = np.tile(np.arange(128, dtype=np.float32), (128, 1)).astype(
        ml_dtypes.bfloat16)
    b1r = np.tile(b1, (128, 1)).astype(np.float32)
    bcombr = np.tile(bcomb, (128, 1)).astype(np.float32)
    bcr = np.tile(bc, (128, 1)).astype(np.float32)

    nc = _get_nc(cfg)
    in_maps = []
    for c in range(cfg.M):
        in_maps.append({
            "xg": xg, "idx": idx_all[c], "ldst": ldst_all[c],
            "val": val_all[c], "deg": deg_all[c], "w1": w1h, "w2c": w2c,
            "b1r": b1r, "bcombr": bcombr, "bcr": bcr, "iota": iota,
        })
    kw = dict(_run_kwargs or {})
    res = run_bass_kernel_spmd(nc, in_maps, core_ids=list(range(cfg.M)), **kw)
    shard = np.concatenate(
        [res.results[c]["out"] for c in range(cfg.M)], axis=0)  # [NPAD, NCLS]
    out = shard[slot_of]
    kernel.last_results = res
    return out.astype(np.float32)


# revision 17
# speedup vs baseline: 1.4678x; 1.0392x over previous
"""GCN node classifier (2x spmm + classifier + log_softmax) on 8 trn2 cores.

Strategy: destination-node 1D sharding with spmm linearity.
  spmm(A, x@W1 + b1) = (A x)@W1 + deg * b1^T      (deg = rowsum of A)
  spmm(A, h@W2 + b2)@Wc = (A h)@(W2 Wc) + deg * (b2 Wc)^T
so the gather tables are the RAW node features (x bf16 for layer 1,
relu-h bf16 for layer 2) — no dense pre-pass over all nodes, and the
layer weights are applied per dst tile after aggregation.

Each core owns 12,800 dst slots (100 tiles x 128 lanes). Host assigns
nodes to slots with a greedy 4-d balancer so that every (src-quarter,
dst-tile) edge bucket fits in KSEG=4 chunks of 128 edges (the int16
gather index forces 4 quarter views of the 102,400-row table). Per-edge
source rows are fetched with GPSIMD dma_gather (256B rows); the
segment-sum is a tensor-engine matmul against per-chunk scatter
matrices V[e, dst_lane] = edge_val[e] built on DVE with
(iota == ldst) * val, accumulated transposed (psT = Xg^T V) so the
per-tile epilogue can feed psT straight back as lhsT for the weight
matmul. log-softmax is fused per tile. Between layers the per-shard
relu-h table is AllGather'ed into a Shared DRAM tensor.
"""

import numpy as np
import ml_dtypes

from contextlib import ExitStack


# ---------------------------------------------------------------- config ---
class Cfg:
    M = 8                 # cores
    N_NODES = 100000
    N_EDGES = 1600000
    IN_DIM = 128
    HID = 64
    NCLS = 40
    NT = 100              # dst tiles per core (128 lanes each)
    KSEG = 4              # chunks (of 128 edges) per (quarter, tile) segment
    SLABC = 25            # chunks per gather slab
    SINGLE_PACKET = False  # multi-packet gathers (single-packet hangs >~1K idxs)
    NQUEUES = 4           # spread gathers over all 4 SWDGE queues
    MSGBUFS = 18
    IDXBUFS = 1
    GE = 10               # tiles per epilogue-matmul batch
    LNG = 20              # tiles per deferred-Ln group
    NZBIAS = False        # set per-input: any of b1/b2/bc nonzero

    @property
    def PADSHARD(self):
        return self.NT * 128

    @property
    def NPAD(self):
        return self.PADSHARD * self.M

    @property
    def QROWS(self):
        return self.NPAD // 4

    @property
    def SEG(self):
        return self.KSEG * 128

    @property
    def CQ(self):
        return self.NT * self.KSEG          # chunks per quarter

    @property
    def NSLAB(self):
        assert self.CQ % self.SLABC == 0
        return self.CQ // self.SLABC        # gather slabs per quarter

    @property
    def CHUNKS(self):
        return 4 * self.CQ


CFG = Cfg()


# ------------------------------------------------------------- host plan ---
def _assign_slots(cfg, edge_row, edge_col):
    """Assign nodes to table slots so every (src-quarter, dst-tile) edge
    bucket holds <= KSEG*128 edges. Returns slot_of[node] -> [0, NPAD).

    Nodes are first split into 4 fixed quarter groups (so each node's
    src-quarter is pinned), then greedily packed into the 2*NT tiles of
    their own quarter balancing the 4-vector of per-src-quarter in-edge
    counts.
    """
    N, NPAD, QROWS, NT, M = cfg.N_NODES, cfg.NPAD, cfg.QROWS, cfg.NT, cfg.M
    TPQ = QROWS // 128                       # tiles per quarter (2 cores)
    rng = np.random.default_rng(12345)
    order = rng.permutation(N)
    qgrp = np.empty(N, dtype=np.int64)       # node -> quarter group
    npq = N // 4
    for q in range(4):
        qgrp[order[q * npq:(q + 1) * npq]] = q
    qgrp[order[4 * npq:]] = 3

    # per-node in-edge count by source quarter
    cnt = np.zeros((N, 4), dtype=np.int64)
    np.add.at(cnt, (edge_row, qgrp[edge_col]), 1)

    slot_of = np.empty(N, dtype=np.int64)
    for q in range(4):
        nodes = np.where(qgrp == q)[0]
        c = cnt[nodes].astype(np.float32)            # [nq, 4]
        tot = c.sum(1)
        o = np.argsort(-tot, kind="stable")
        nodes, c = nodes[o], c[o]
        loads = np.zeros((TPQ, 4), dtype=np.float32)
        fill = np.zeros(TPQ, dtype=np.int64)
        pos = np.empty(nodes.size, dtype=np.int64)
        for i in range(nodes.size):
            cand = np.max(loads + c[i], axis=1) + (fill >= 128) * 1e9
            b = int(np.argmin(cand))
            loads[b] += c[i]
            pos[i] = b * 128 + fill[b]
            fill[b] += 1
        slot_of[nodes] = q * QROWS + pos
    return slot_of


def _plan(cfg, edge_row, edge_col, edge_val, slot_of):
    """Bucket/sort/pad edges per core. Returns per-core arrays:
    idx16 [128, 4*CQ*128/16] int16, ldstT/valT [128, CHUNKS] bf16,
    plus degs [128, NT] f32 per core.
    """
    M, NT, KSEG, SEG, CQ, QROWS = cfg.M, cfg.NT, cfg.KSEG, cfg.SEG, cfg.CQ, cfg.QROWS
    PADSHARD = cfg.PADSHARD

    src_slot = slot_of[edge_col]
    dst_slot = slot_of[edge_row]
    q_of = src_slot // QROWS
    i_of = src_slot % QROWS
    core_of = dst_slot // PADSHARD
    dloc = dst_slot % PADSHARD
    t_of = dloc // 128
    l_of = dloc % 128

    deg = np.zeros(cfg.NPAD, dtype=np.float64)
    np.add.at(deg, dst_slot, edge_val.astype(np.float64))

    L = 4 * CQ * 128
    idx_all, ldst_all, val_all, deg_all = [], [], [], []
    for c in range(M):
        sel = core_of == c
        segid = q_of[sel] * NT + t_of[sel]
        order = np.argsort(segid, kind="stable")
        sid = segid[order]
        idx_s = i_of[sel][order]
        l_s = l_of[sel][order]
        v_s = edge_val[sel][order]

        counts = np.bincount(sid, minlength=4 * NT)
        if counts.max() > SEG:
            raise ValueError(f"segment overflow: {counts.max()} > {SEG}")
        starts = np.arange(4 * NT) * SEG
        pos = starts[sid] + (np.arange(sid.size)
                             - np.concatenate(([0], np.cumsum(counts)))[sid])

        idx = np.zeros(L, dtype=np.int16)
        ldst = np.zeros(L, dtype=np.float32)
        val = np.zeros(L, dtype=np.float32)
        idx[pos] = idx_s.astype(np.int16)
        ldst[pos] = l_s.astype(np.float32)
        val[pos] = v_s.astype(np.float32)

        # wrap indices: idx i -> [i%16, i//16], replicated on all 8 q7 cores
        idxw = np.tile(idx.reshape(-1, 16).T, (8, 1)).copy()
        ldstT = np.ascontiguousarray(ldst.reshape(-1, 128).T)
        valT = np.ascontiguousarray(val.reshape(-1, 128).T)
        degs = np.ascontiguousarray(
            deg[c * PADSHARD:(c + 1) * PADSHARD].reshape(NT, 128).T
        ).astype(np.float32)
        idx_all.append(idxw)
        ldst_all.append(ldstT)
        val_all.append(valT)
        deg_all.append(degs)
    return idx_all, ldst_all, val_all, deg_all


# --------------------------------------------------------- device program ---
def _build(cfg, timing=False, nzbias=False):
    import os
    from concourse import bacc, tile
    import concourse.mybir as mybir
    kdbg = bool(os.environ.get("KDBG"))

    f32 = mybir.dt.float32
    bf16 = mybir.dt.bfloat16
    i16 = mybir.dt.int16
    AOP = mybir.AluOpType
    ACT = mybir.ActivationFunctionType

    nc = bacc.Bacc("TRN2", target_bir_lowering=False, debug=False,
                   num_devices=1 if timing else cfg.M,
                   dynamic_dma_scratch_size=16384,
                   num_swdge_queues=cfg.NQUEUES)

    NPAD, QROWS, NT, KSEG, CQ, SLABC, NSLAB = (
        cfg.NPAD, cfg.QROWS, cfg.NT, cfg.KSEG, cfg.CQ, cfg.SLABC, cfg.NSLAB)
    CHUNKS, HID, NCLS, IN_DIM = cfg.CHUNKS, cfg.HID, cfg.NCLS, cfg.IN_DIM
    LQ16 = CQ * 128 // 16              # idx columns per quarter
    SLAB16 = SLABC * 128 // 16         # idx columns per slab

    # -------- I/O
    XG = nc.dram_tensor("xg", [NPAD, IN_DIM], bf16, kind="ExternalInput")
    IDX = nc.dram_tensor("idx", [128, 4 * LQ16], i16, kind="ExternalInput")
    LDST = nc.dram_tensor("ldst", [128, CHUNKS], f32, kind="ExternalInput")
    VAL = nc.dram_tensor("val", [128, CHUNKS], f32, kind="ExternalInput")
    DEG = nc.dram_tensor("deg", [128, NT], f32, kind="ExternalInput")
    W1 = nc.dram_tensor("w1", [IN_DIM, HID], bf16, kind="ExternalInput")
    W2C = nc.dram_tensor("w2c", [HID, NCLS], bf16, kind="ExternalInput")
    B1R = nc.dram_tensor("b1r", [128, HID], f32, kind="ExternalInput")
    BCOMBR = nc.dram_tensor("bcombr", [128, NCLS], f32, kind="ExternalInput")
    BCR = nc.dram_tensor("bcr", [128, NCLS], f32, kind="ExternalInput")
    IOTA = nc.dram_tensor("iota", [128, 128], bf16, kind="ExternalInput")
    OUT = nc.dram_tensor("out", [cfg.PADSHARD, NCLS], f32, kind="ExternalOutput")
    HDBG = (nc.dram_tensor("hdbg", [cfg.PADSHARD, HID], bf16,
                           kind="ExternalOutput") if kdbg else None)
    LDBG = (nc.dram_tensor("ldbg", [128, NT * NCLS], f32,
                           kind="ExternalOutput") if kdbg else None)
    SMDBG = (nc.dram_tensor("smdbg", [128, NT], f32,
                            kind="ExternalOutput") if kdbg else None)

    # -------- internal DRAM
    HS = nc.dram_tensor("hshard", [cfg.PADSHARD, 128], bf16)    # cols 64+: junk
    HF = nc.dram_tensor("hfull", [NPAD, 128], bf16, addr_space="Shared")

    with tile.TileContext(nc) as tc, ExitStack() as top:
        cpool = top.enter_context(tc.tile_pool(name="consts", bufs=1))
        w1s = cpool.tile([IN_DIM, HID], bf16)
        nc.sync.dma_start(out=w1s, in_=W1[:, :])
        w2cs = cpool.tile([HID, NCLS], bf16)
        nc.sync.dma_start(out=w2cs, in_=W2C[:, :])
        b1rs = cpool.tile([128, HID], f32)
        nc.sync.dma_start(out=b1rs, in_=B1R[:, :])
        bcombs = cpool.tile([128, NCLS], f32)
        nc.sync.dma_start(out=bcombs, in_=BCOMBR[:, :])
        bcrs = cpool.tile([128, NCLS], f32)
        nc.sync.dma_start(out=bcrs, in_=BCR[:, :])
        iot = cpool.tile([128, 128], bf16)
        nc.sync.dma_start(out=iot, in_=IOTA[:, :])
        degs = cpool.tile([128, NT], f32)
        nc.sync.dma_start(out=degs, in_=DEG[:, :])

        edg = top.enter_context(tc.tile_pool(name="edg", bufs=1))
        ldsts = edg.tile([128, CHUNKS], f32)
        nc.sync.dma_start(out=ldsts, in_=LDST[:, :])
        vals = edg.tile([128, CHUNKS], f32)
        nc.sync.dma_start(out=vals, in_=VAL[:, :])

        # idx tiles for BOTH layers, loaded up front
        idxp = top.enter_context(tc.tile_pool(name="idxp", bufs=1))
        its = []
        for li in range(2):
            row = []
            for q in range(4):
                it = idxp.tile([128, LQ16], i16, tag=f"idx{li}_{q}",
                               name=f"idx{li}_{q}")
                nc.sync.dma_start(out=it, in_=IDX[:, q * LQ16:(q + 1) * LQ16])
                row.append(it)
            its.append(row)

        # ============ spmm layer runner: per-tile single psum group across
        # all 4 quarters, accumulating transposed (psT = Xg^T V); epilogue
        # split into a per-tile part (cast) and a batched per-GE-tiles part
        # (weight matmuls etc) to keep the PE stream free of cross-engine
        # round trips.
        def spmm_layer(tab, width, lits, epi_tile, epi_group, pools):
            msg, vp, psb = pools
            GEB = cfg.GE
            slabs = [[None] * NSLAB for _ in range(4)]

            def ensure_slab(q, s):
                if slabs[q][s] is None:
                    mt = msg.tile([128, SLABC, 128], bf16)
                    nc.gpsimd.dma_gather(
                        mt, tab[q * QROWS:(q + 1) * QROWS, :],
                        lits[q][:, s * SLAB16:(s + 1) * SLAB16],
                        num_idxs=SLABC * 128, num_idxs_reg=SLABC * 128,
                        elem_size=128, elem_step=128,
                        single_packet=cfg.SINGLE_PACKET,
                        queue_num=(q * NSLAB + s) % cfg.NQUEUES)
                    slabs[q][s] = mt
                return slabs[q][s]

            for t in range(NT):
                ps = psb.tile([width, 128], f32)
                for q in range(4):
                    j0 = t * KSEG
                    vt = vp.tile([128, KSEG, 128], bf16)
                    for k in range(KSEG):
                        gj = q * CQ + j0 + k             # global chunk
                        nc.vector.tensor_scalar(
                            vt[:, k, :], iot, ldsts[:, gj:gj + 1],
                            vals[:, gj:gj + 1], AOP.is_equal, AOP.mult)
                    for k in range(KSEG):
                        j = j0 + k                       # chunk in quarter
                        mt = ensure_slab(q, j // SLABC)
                        nc.tensor.matmul(ps, lhsT=mt[:, j % SLABC, 0:width],
                                         rhs=vt[:, k, :],
                                         start=(q == 0 and k == 0),
                                         stop=(q == 3 and k == KSEG - 1))
                epi_tile(t, ps)
                if t % GEB == GEB - 1:
                    epi_group(t - GEB + 1, GEB)

        # ================= layer 1: h = relu((A x)@W1 + deg*b1^T), store bf16
        with tc.tile_pool(name="msg", bufs=cfg.MSGBUFS) as msg, \
             tc.tile_pool(name="vp", bufs=8) as vp, \
             tc.tile_pool(name="psb", bufs=3, space="PSUM") as psb, \
             tc.tile_pool(name="tc1", bufs=cfg.GE + 2) as tp1, \
             tc.tile_pool(name="tc2", bufs=3) as tp2, \
             tc.tile_pool(name="pse", bufs=3, space="PSUM") as pse:
            pss1 = {}

            def epi1_tile(t, ps):
                pss = tp1.tile([IN_DIM, 128], bf16, tag="pss", name="pss")
                nc.scalar.activation(pss, ps, ACT.Copy)
                pss1[t] = pss

            def epi1_group(t0, n):
                for t in range(t0, t0 + n):
                    ph = pse.tile([128, HID], f32)
                    nc.tensor.matmul(ph, lhsT=pss1.pop(t), rhs=w1s,
                                     start=True, stop=True)
                    ht = tp2.tile([128, HID], bf16, tag="ht")
                    if nzbias:
                        tb = tp2.tile([128, HID], f32, tag="tb")
                        nc.vector.tensor_scalar(tb, b1rs, degs[:, t:t + 1],
                                                None, AOP.mult)
                        hsum = tp2.tile([128, HID], f32, tag="hsum")
                        nc.vector.tensor_tensor(hsum, ph, tb, AOP.add)
                        nc.scalar.activation(ht, hsum, ACT.Relu)
                    else:
                        nc.scalar.activation(ht, ph, ACT.Relu)
                    nc.sync.dma_start(out=HS[t * 128:(t + 1) * 128, 0:HID],
                                      in_=ht)

            spmm_layer(XG, IN_DIM, its[0], epi1_tile, epi1_group,
                       (msg, vp, psb))
            if kdbg:
                nc.sync.dma_start(out=HDBG[:, :], in_=HS[:, 0:HID])
            if not timing:
                nc.gpsimd.collective_compute(
                    "AllGather", mybir.AluOpType.bypass,
                    replica_groups=[list(range(cfg.M))],
                    ins=[HS[:, :]], outs=[HF[:, :]])

        # ================= layer 2 + fused classifier/log_softmax
        with tc.tile_pool(name="msg2", bufs=cfg.MSGBUFS) as msg2, \
             tc.tile_pool(name="vp2", bufs=8) as vp2, \
             tc.tile_pool(name="psb2", bufs=3, space="PSUM") as psb2, \
             tc.tile_pool(name="te1", bufs=cfg.GE + 2) as te1, \
             tc.tile_pool(name="te2", bufs=3) as te2, \
             tc.tile_pool(name="te3", bufs=2) as te3, \
             tc.tile_pool(name="psf", bufs=3, space="PSUM") as psf:
            G = cfg.LNG
            assert NT % G == 0 and G % cfg.GE == 0
            pss2 = {}
            state = {}

            def epi2_tile(t, ps):
                pss = te1.tile([HID, 128], bf16, tag="pss", name="pss")
                nc.scalar.activation(pss, ps, ACT.Copy)
                pss2[t] = pss

            def epi2_group(t0, n):
                for t in range(t0, t0 + n):
                    g, i = t // G, t % G
                    if i == 0:
                        state["lgg"] = te3.tile([128, G, NCLS], f32,
                                                tag="lgg", name="lgg")
                        state["negg"] = te3.tile([128, G], f32,
                                                 tag="negg", name="negg")
                        state["smg"] = te3.tile([128, G], f32,
                                                tag="smg", name="smg")
                    lgg, negg, smg = state["lgg"], state["negg"], state["smg"]
                    psl = psf.tile([128, NCLS], f32)
                    nc.tensor.matmul(psl, lhsT=pss2.pop(t), rhs=w2cs,
                                     start=True, stop=True)
                    if nzbias:
                        tb = te2.tile([128, NCLS], f32, tag="tb")
                        nc.vector.tensor_scalar(tb, bcombs, degs[:, t:t + 1],
                                                None, AOP.mult)
                        lg0 = te2.tile([128, NCLS], f32, tag="lg0")
                        nc.vector.tensor_tensor(lg0, psl, tb, AOP.add)
                        nc.gpsimd.tensor_tensor(lgg[:, i, :], lg0, bcrs,
                                                AOP.add)
                    else:
                        nc.scalar.activation(lgg[:, i, :], psl, ACT.Copy)
                    if i == G - 1:
                        if kdbg:
                            nc.sync.dma_start(
                                out=LDBG[:, g * G * NCLS:(g + 1) * G * NCLS],
                                in_=lgg.rearrange("p a b -> p (a b)"))
                            nc.sync.dma_start(out=SMDBG[:, g * G:(g + 1) * G],
                                              in_=smg)
                        nc.vector.tensor_reduce(negg[:, :], lgg,
                                                mybir.AxisListType.X, AOP.max,
                                                negate=True)
                        for ii in range(G):
                            et = te2.tile([128, NCLS], f32, tag="et")
                            nc.scalar.activation(et, lgg[:, ii, :], ACT.Exp,
                                                 bias=negg[:, ii:ii + 1],
                                                 accum_out=smg[:, ii:ii + 1])
                        lng = te2.tile([128, G], f32, tag="lng")
                        nc.scalar.activation(lng, smg, ACT.Ln)
                        shg = te2.tile([128, G], f32, tag="shg")
                        nc.vector.tensor_tensor(shg, negg, lng, AOP.subtract)
                        for ii in range(G):
                            tt = g * G + ii
                            ot = te2.tile([128, NCLS], f32, tag="ot")
                            nc.vector.tensor_scalar(ot, lgg[:, ii, :],
                                                    shg[:, ii:ii + 1], None,
                                                    AOP.add)
                            nc.sync.dma_start(
                                out=OUT[tt * 128:(tt + 1) * 128, :], in_=ot)

            spmm_layer(HF, HID, its[1], epi2_tile, epi2_group,
                       (msg2, vp2, psb2))

    nc.compile()
    return nc


_NC_CACHE = {}


def _get_nc(cfg):
    key = (cfg.NT, cfg.KSEG, cfg.SLABC, cfg.NZBIAS)
    if key not in _NC_CACHE:
        _NC_CACHE[key] = _build(cfg, nzbias=cfg.NZBIAS)
    return _NC_CACHE[key]


# ------------------------------------------------------------------ main ---
def kernel(x, edge_row, edge_col, edge_val, W1, b1, W2, b2, Wc, bc,
           _run_kwargs=None):
    from concourse.bass_utils import run_bass_kernel_spmd

    cfg = CFG
    x = np.asarray(x, dtype=np.float32)
    edge_row = np.asarray(edge_row, dtype=np.int64)
    edge_col = np.asarray(edge_col, dtype=np.int64)
    edge_val = np.asarray(edge_val, dtype=np.float32)
    W1 = np.asarray(W1, dtype=np.float32)
    W2 = np.asarray(W2, dtype=np.float32)
    Wc = np.asarray(Wc, dtype=np.float32)
    b1 = np.asarray(b1, dtype=np.float32)
    b2 = np.asarray(b2, dtype=np.float32)
    bc = np.asarray(bc, dtype=np.float32)

    cfg.NZBIAS = bool(np.any(b1) or np.any(b2) or np.any(bc))
    slot_of = _assign_slots(cfg, edge_row, edge_col)
    try:
        idx_all, ldst_all, val_all, deg_all = _plan(
            cfg, edge_row, edge_col, edge_val, slot_of)
    except ValueError:
        cfg.KSEG += 1
        idx_all, ldst_all, val_all, deg_all = _plan(
            cfg, edge_row, edge_col, edge_val, slot_of)

    xg = np.zeros((cfg.NPAD, cfg.IN_DIM), dtype=ml_dtypes.bfloat16)
    xg[slot_of] = x.astype(ml_dtypes.bfloat16)

    w1h = W1.astype(ml_dtypes.bfloat16)
    w2c = (W2 @ Wc).astype(ml_dtypes.bfloat16)
    bcomb = b2 @ Wc
    iota = np.tile(np.arange(128, dtype=np.float32), (128, 1)).astype(
        ml_dtypes.bfloat16)
    b1r = np.tile(b1, (128, 1)).astype(np.float32)
    bcombr = np.tile(bcomb, (128, 1)).astype(np.float32)
    bcr = np.tile(bc, (128, 1)).astype(np.float32)

    nc = _get_nc(cfg)
    in_maps = []
    for c in range(cfg.M):
        in_maps.append({
            "xg": xg, "idx": idx_all[c], "ldst": ldst_all[c],
            "val": val_all[c], "deg": deg_all[c], "w1": w1h, "w2c": w2c,
            "b1r": b1r, "bcombr": bcombr, "bcr": bcr, "iota": iota,
        })
    kw = dict(_run_kwargs or {})
    res = run_bass_kernel_spmd(nc, in_maps, core_ids=list(range(cfg.M)), **kw)
    shard = np.concatenate(
        [res.results[c]["out"] for c in range(cfg.M)], axis=0)  # [NPAD, NCLS]
    out = shard[slot_of]
    kernel.last_results = res
    return out.astype(np.float32)


# revision 21
# speedup vs baseline: 1.5797x; 1.0763x over previous
"""GCN node classifier (2x spmm + classifier + log_softmax) on 8 trn2 cores.

Strategy: destination-node 1D sharding with spmm linearity.
  spmm(A, x@W1 + b1) = (A x)@W1 + deg * b1^T      (deg = rowsum of A)
  spmm(A, h@W2 + b2)@Wc = (A h)@(W2 Wc) + deg * (b2 Wc)^T
so the gather tables are the RAW node features (x bf16 for layer 1,
relu-h bf16 for layer 2) — no dense pre-pass over all nodes, and the
layer weights are applied per dst tile after aggregation.

Each core owns 12,800 dst slots (100 tiles x 128 lanes). Host assigns
nodes to slots with a greedy 4-d balancer so that every (src-quarter,
dst-tile) edge bucket fits in KSEG=4 chunks of 128 edges (the int16
gather index forces 4 quarter views of the 102,400-row table). Per-edge
source rows are fetched with GPSIMD dma_gather (256B rows); the
segment-sum is a tensor-engine matmul against per-chunk scatter
matrices V[e, dst_lane] = edge_val[e] built on DVE with
(iota == ldst) * val, accumulated transposed (psT = Xg^T V) so the
per-tile epilogue can feed psT straight back as lhsT for the weight
matmul. log-softmax is fused per tile. Between layers the per-shard
relu-h table is AllGather'ed into a Shared DRAM tensor.
"""

import numpy as np
import ml_dtypes

from contextlib import ExitStack


# ---------------------------------------------------------------- config ---
class Cfg:
    M = 8                 # cores
    N_NODES = 100000
    N_EDGES = 1600000
    IN_DIM = 128
    HID = 64
    NCLS = 40
    NT = 100              # dst tiles per core (128 lanes each)
    KSEG = 4              # chunks (of 128 edges) per (quarter, tile) segment
    SLABC = 25            # chunks per gather slab
    SINGLE_PACKET = False  # multi-packet gathers (single-packet hangs >~1K idxs)
    NQUEUES = 4           # spread gathers over all 4 SWDGE queues
    MSGBUFS = 13
    IDXBUFS = 1
    GE = 10               # tiles per epilogue-matmul batch
    LNG = 20              # tiles per deferred-Ln group
    NZBIAS = False        # set per-input: any of b1/b2/bc nonzero

    @property
    def PADSHARD(self):
        return self.NT * 128

    @property
    def NPAD(self):
        return self.PADSHARD * self.M

    @property
    def QROWS(self):
        return self.NPAD // 4

    @property
    def SEG(self):
        return self.KSEG * 128

    @property
    def CQ(self):
        return self.NT * self.KSEG          # chunks per quarter

    @property
    def NSLAB(self):
        assert self.CQ % self.SLABC == 0
        return self.CQ // self.SLABC        # gather slabs per quarter

    @property
    def CHUNKS(self):
        return 4 * self.CQ


CFG = Cfg()


# ------------------------------------------------------------- host plan ---
def _assign_slots(cfg, edge_row, edge_col):
    """Assign nodes to table slots so every (src-quarter, dst-tile) edge
    bucket holds <= KSEG*128 edges. Returns slot_of[node] -> [0, NPAD).

    Nodes are first split into 4 fixed quarter groups (so each node's
    src-quarter is pinned), then greedily packed into the 2*NT tiles of
    their own quarter balancing the 4-vector of per-src-quarter in-edge
    counts.
    """
    N, NPAD, QROWS, NT, M = cfg.N_NODES, cfg.NPAD, cfg.QROWS, cfg.NT, cfg.M
    TPQ = QROWS // 128                       # tiles per quarter (2 cores)
    rng = np.random.default_rng(12345)
    order = rng.permutation(N)
    qgrp = np.empty(N, dtype=np.int64)       # node -> quarter group
    npq = N // 4
    for q in range(4):
        qgrp[order[q * npq:(q + 1) * npq]] = q
    qgrp[order[4 * npq:]] = 3

    # per-node in-edge count by source quarter
    cnt = np.zeros((N, 4), dtype=np.int64)
    np.add.at(cnt, (edge_row, qgrp[edge_col]), 1)

    slot_of = np.empty(N, dtype=np.int64)
    for q in range(4):
        nodes = np.where(qgrp == q)[0]
        c = cnt[nodes].astype(np.float32)            # [nq, 4]
        tot = c.sum(1)
        o = np.argsort(-tot, kind="stable")
        nodes, c = nodes[o], c[o]
        loads = np.zeros((TPQ, 4), dtype=np.float32)
        fill = np.zeros(TPQ, dtype=np.int64)
        pos = np.empty(nodes.size, dtype=np.int64)
        for i in range(nodes.size):
            cand = np.max(loads + c[i], axis=1) + (fill >= 128) * 1e9
            b = int(np.argmin(cand))
            loads[b] += c[i]
            pos[i] = b * 128 + fill[b]
            fill[b] += 1
        slot_of[nodes] = q * QROWS + pos
    return slot_of


def _plan(cfg, edge_row, edge_col, edge_val, slot_of):
    """Bucket/sort/pad edges per core. Returns per-core arrays:
    idx16 [128, 4*CQ*128/16] int16, ldstT/valT [128, CHUNKS] bf16,
    plus degs [128, NT] f32 per core.
    """
    M, NT, KSEG, SEG, CQ, QROWS = cfg.M, cfg.NT, cfg.KSEG, cfg.SEG, cfg.CQ, cfg.QROWS
    PADSHARD = cfg.PADSHARD

    src_slot = slot_of[edge_col]
    dst_slot = slot_of[edge_row]
    q_of = src_slot // QROWS
    i_of = src_slot % QROWS
    core_of = dst_slot // PADSHARD
    dloc = dst_slot % PADSHARD
    t_of = dloc // 128
    l_of = dloc % 128

    deg = np.zeros(cfg.NPAD, dtype=np.float64)
    np.add.at(deg, dst_slot, edge_val.astype(np.float64))

    L = 4 * CQ * 128
    idx_all, ldst_all, val_all, deg_all = [], [], [], []
    for c in range(M):
        sel = core_of == c
        segid = q_of[sel] * NT + t_of[sel]
        order = np.argsort(segid, kind="stable")
        sid = segid[order]
        idx_s = i_of[sel][order]
        l_s = l_of[sel][order]
        v_s = edge_val[sel][order]

        counts = np.bincount(sid, minlength=4 * NT)
        if counts.max() > SEG:
            raise ValueError(f"segment overflow: {counts.max()} > {SEG}")
        starts = np.arange(4 * NT) * SEG
        pos = starts[sid] + (np.arange(sid.size)
                             - np.concatenate(([0], np.cumsum(counts)))[sid])

        idx = np.zeros(L, dtype=np.int16)
        ldst = np.zeros(L, dtype=np.float32)
        val = np.zeros(L, dtype=np.float32)
        idx[pos] = idx_s.astype(np.int16)
        ldst[pos] = l_s.astype(np.float32)
        val[pos] = v_s.astype(np.float32)

        # wrap indices: idx i -> [i%16, i//16], replicated on all 8 q7 cores
        idxw = np.tile(idx.reshape(-1, 16).T, (8, 1)).copy()
        ldstT = np.ascontiguousarray(ldst.reshape(-1, 128).T)
        valT = np.ascontiguousarray(val.reshape(-1, 128).T)
        degs = np.ascontiguousarray(
            deg[c * PADSHARD:(c + 1) * PADSHARD].reshape(NT, 128).T
        ).astype(np.float32)
        idx_all.append(idxw)
        ldst_all.append(ldstT)
        val_all.append(valT)
        deg_all.append(degs)
    return idx_all, ldst_all, val_all, deg_all


# --------------------------------------------------------- device program ---
def _build(cfg, timing=False, nzbias=False):
    import os
    from concourse import bacc, tile
    import concourse.mybir as mybir
    kdbg = bool(os.environ.get("KDBG"))

    f32 = mybir.dt.float32
    bf16 = mybir.dt.bfloat16
    i16 = mybir.dt.int16
    AOP = mybir.AluOpType
    ACT = mybir.ActivationFunctionType

    nc = bacc.Bacc("TRN2", target_bir_lowering=False, debug=False,
                   num_devices=1 if timing else cfg.M,
                   dynamic_dma_scratch_size=16384,
                   num_swdge_queues=cfg.NQUEUES)

    NPAD, QROWS, NT, KSEG, CQ, SLABC, NSLAB = (
        cfg.NPAD, cfg.QROWS, cfg.NT, cfg.KSEG, cfg.CQ, cfg.SLABC, cfg.NSLAB)
    CHUNKS, HID, NCLS, IN_DIM = cfg.CHUNKS, cfg.HID, cfg.NCLS, cfg.IN_DIM
    LQ16 = CQ * 128 // 16              # idx columns per quarter
    SLAB16 = SLABC * 128 // 16         # idx columns per slab

    # -------- I/O
    XG = nc.dram_tensor("xg", [NPAD, IN_DIM], bf16, kind="ExternalInput")
    IDX = nc.dram_tensor("idx", [128, 4 * LQ16], i16, kind="ExternalInput")
    LDST = nc.dram_tensor("ldst", [128, CHUNKS], f32, kind="ExternalInput")
    VAL = nc.dram_tensor("val", [128, CHUNKS], f32, kind="ExternalInput")
    DEG = nc.dram_tensor("deg", [128, NT], f32, kind="ExternalInput")
    W1 = nc.dram_tensor("w1", [IN_DIM, HID], bf16, kind="ExternalInput")
    W2C = nc.dram_tensor("w2c", [HID, NCLS], bf16, kind="ExternalInput")
    B1R = nc.dram_tensor("b1r", [128, HID], f32, kind="ExternalInput")
    BCOMBR = nc.dram_tensor("bcombr", [128, NCLS], f32, kind="ExternalInput")
    BCR = nc.dram_tensor("bcr", [128, NCLS], f32, kind="ExternalInput")
    IOTA = nc.dram_tensor("iota", [128, 128], bf16, kind="ExternalInput")
    OUT = nc.dram_tensor("out", [cfg.PADSHARD, NCLS], f32, kind="ExternalOutput")
    HDBG = (nc.dram_tensor("hdbg", [cfg.PADSHARD, HID], bf16,
                           kind="ExternalOutput") if kdbg else None)
    LDBG = (nc.dram_tensor("ldbg", [128, NT * NCLS], f32,
                           kind="ExternalOutput") if kdbg else None)
    SMDBG = (nc.dram_tensor("smdbg", [128, NT], f32,
                            kind="ExternalOutput") if kdbg else None)

    # -------- internal DRAM
    HS = nc.dram_tensor("hshard", [cfg.PADSHARD, 128], bf16)    # cols 64+: junk
    HF = nc.dram_tensor("hfull", [NPAD, 128], bf16, addr_space="Shared")

    with tile.TileContext(nc) as tc, ExitStack() as top:
        cpool = top.enter_context(tc.tile_pool(name="consts", bufs=1))
        w1s = cpool.tile([IN_DIM, HID], bf16)
        nc.sync.dma_start(out=w1s, in_=W1[:, :])
        w2cs = cpool.tile([HID, NCLS], bf16)
        nc.sync.dma_start(out=w2cs, in_=W2C[:, :])
        b1rs = cpool.tile([128, HID], f32)
        nc.sync.dma_start(out=b1rs, in_=B1R[:, :])
        bcombs = cpool.tile([128, NCLS], f32)
        nc.sync.dma_start(out=bcombs, in_=BCOMBR[:, :])
        bcrs = cpool.tile([128, NCLS], f32)
        nc.sync.dma_start(out=bcrs, in_=BCR[:, :])
        iot = cpool.tile([128, 128], bf16)
        nc.sync.dma_start(out=iot, in_=IOTA[:, :])
        degs = cpool.tile([128, NT], f32)
        nc.sync.dma_start(out=degs, in_=DEG[:, :])

        edg = top.enter_context(tc.tile_pool(name="edg", bufs=1))
        ldsts = edg.tile([128, CHUNKS], f32)
        nc.sync.dma_start(out=ldsts, in_=LDST[:, :])
        vals = edg.tile([128, CHUNKS], f32)
        nc.sync.dma_start(out=vals, in_=VAL[:, :])

        # idx tiles for BOTH layers, loaded up front
        idxp = top.enter_context(tc.tile_pool(name="idxp", bufs=1))
        its = []
        for li in range(2):
            row = []
            for q in range(4):
                it = idxp.tile([128, LQ16], i16, tag=f"idx{li}_{q}",
                               name=f"idx{li}_{q}")
                nc.sync.dma_start(out=it, in_=IDX[:, q * LQ16:(q + 1) * LQ16])
                row.append(it)
            its.append(row)

        # ============ spmm layer runner: per-tile single psum group across
        # all 4 quarters, accumulating transposed (psT = Xg^T V); epilogue
        # split into a per-tile part (cast) and a batched per-GE-tiles part
        # (weight matmuls etc) to keep the PE stream free of cross-engine
        # round trips.
        def spmm_layer(tab, width, lits, epi_tile, epi_group, pools):
            msg, vp, psb = pools
            GEB = cfg.GE
            slabs = [[None] * NSLAB for _ in range(4)]

            def ensure_slab(q, s):
                if slabs[q][s] is None:
                    mt = msg.tile([128, SLABC, 128], bf16)
                    nc.gpsimd.dma_gather(
                        mt, tab[q * QROWS:(q + 1) * QROWS, :],
                        lits[q][:, s * SLAB16:(s + 1) * SLAB16],
                        num_idxs=SLABC * 128, num_idxs_reg=SLABC * 128,
                        elem_size=128, elem_step=128,
                        single_packet=cfg.SINGLE_PACKET,
                        queue_num=(q * NSLAB + s) % cfg.NQUEUES)
                    slabs[q][s] = mt
                return slabs[q][s]

            for t in range(NT):
                ps = psb.tile([width, 128], f32)
                for q in range(4):
                    j0 = t * KSEG
                    vt = vp.tile([128, KSEG, 128], bf16)
                    veng = nc.gpsimd if q == 3 else nc.vector
                    for k in range(KSEG):
                        gj = q * CQ + j0 + k             # global chunk
                        veng.tensor_scalar(
                            vt[:, k, :], iot, ldsts[:, gj:gj + 1],
                            vals[:, gj:gj + 1], AOP.is_equal, AOP.mult)
                    for k in range(KSEG):
                        j = j0 + k                       # chunk in quarter
                        mt = ensure_slab(q, j // SLABC)
                        nc.tensor.matmul(ps, lhsT=mt[:, j % SLABC, 0:width],
                                         rhs=vt[:, k, :],
                                         start=(q == 0 and k == 0),
                                         stop=(q == 3 and k == KSEG - 1))
                epi_tile(t, ps)
                if t % GEB == GEB - 1:
                    epi_group(t - GEB + 1, GEB)

        # ================= layer 1: h = relu((A x)@W1 + deg*b1^T), store bf16
        with tc.tile_pool(name="msg", bufs=cfg.MSGBUFS) as msg, \
             tc.tile_pool(name="vp", bufs=8) as vp, \
             tc.tile_pool(name="psb", bufs=3, space="PSUM") as psb, \
             tc.tile_pool(name="tc1", bufs=cfg.GE + 2) as tp1, \
             tc.tile_pool(name="tc2", bufs=3) as tp2, \
             tc.tile_pool(name="pse", bufs=3, space="PSUM") as pse:
            pss1 = {}

            def epi1_tile(t, ps):
                pss = tp1.tile([IN_DIM, 128], bf16, tag="pss", name="pss")
                nc.scalar.activation(pss, ps, ACT.Copy)
                pss1[t] = pss

            def epi1_group(t0, n):
                for t in range(t0, t0 + n):
                    ph = pse.tile([128, HID], f32)
                    nc.tensor.matmul(ph, lhsT=pss1.pop(t), rhs=w1s,
                                     start=True, stop=True)
                    ht = tp2.tile([128, HID], bf16, tag="ht")
                    if nzbias:
                        tb = tp2.tile([128, HID], f32, tag="tb")
                        nc.vector.tensor_scalar(tb, b1rs, degs[:, t:t + 1],
                                                None, AOP.mult)
                        hsum = tp2.tile([128, HID], f32, tag="hsum")
                        nc.vector.tensor_tensor(hsum, ph, tb, AOP.add)
                        nc.scalar.activation(ht, hsum, ACT.Relu)
                    else:
                        nc.scalar.activation(ht, ph, ACT.Relu)
                    nc.sync.dma_start(out=HS[t * 128:(t + 1) * 128, 0:HID],
                                      in_=ht)

            spmm_layer(XG, IN_DIM, its[0], epi1_tile, epi1_group,
                       (msg, vp, psb))
            if kdbg:
                nc.sync.dma_start(out=HDBG[:, :], in_=HS[:, 0:HID])
            if not timing:
                nc.gpsimd.collective_compute(
                    "AllGather", mybir.AluOpType.bypass,
                    replica_groups=[list(range(cfg.M))],
                    ins=[HS[:, :]], outs=[HF[:, :]])

        # ================= layer 2 + fused classifier/log_softmax
        with tc.tile_pool(name="msg2", bufs=cfg.MSGBUFS) as msg2, \
             tc.tile_pool(name="vp2", bufs=8) as vp2, \
             tc.tile_pool(name="psb2", bufs=3, space="PSUM") as psb2, \
             tc.tile_pool(name="te1", bufs=cfg.GE + 2) as te1, \
             tc.tile_pool(name="te2", bufs=3) as te2, \
             tc.tile_pool(name="te3", bufs=2) as te3, \
             tc.tile_pool(name="psf", bufs=3, space="PSUM") as psf:
            G = cfg.LNG
            assert NT % G == 0 and G % cfg.GE == 0
            pss2 = {}
            state = {}

            def epi2_tile(t, ps):
                pss = te1.tile([HID, 128], bf16, tag="pss", name="pss")
                nc.scalar.activation(pss, ps, ACT.Copy)
                pss2[t] = pss

            def epi2_group(t0, n):
                for t in range(t0, t0 + n):
                    g, i = t // G, t % G
                    if i == 0:
                        state["lgg"] = te3.tile([128, G, NCLS], f32,
                                                tag="lgg", name="lgg")
                        state["negg"] = te3.tile([128, G], f32,
                                                 tag="negg", name="negg")
                        state["smg"] = te3.tile([128, G], f32,
                                                tag="smg", name="smg")
                    lgg, negg, smg = state["lgg"], state["negg"], state["smg"]
                    psl = psf.tile([128, NCLS], f32)
                    nc.tensor.matmul(psl, lhsT=pss2.pop(t), rhs=w2cs,
                                     start=True, stop=True)
                    if nzbias:
                        tb = te2.tile([128, NCLS], f32, tag="tb")
                        nc.vector.tensor_scalar(tb, bcombs, degs[:, t:t + 1],
                                                None, AOP.mult)
                        lg0 = te2.tile([128, NCLS], f32, tag="lg0")
                        nc.vector.tensor_tensor(lg0, psl, tb, AOP.add)
                        nc.gpsimd.tensor_tensor(lgg[:, i, :], lg0, bcrs,
                                                AOP.add)
                    else:
                        nc.scalar.activation(lgg[:, i, :], psl, ACT.Copy)
                    if i == G - 1:
                        if kdbg:
                            nc.sync.dma_start(
                                out=LDBG[:, g * G * NCLS:(g + 1) * G * NCLS],
                                in_=lgg.rearrange("p a b -> p (a b)"))
                            nc.sync.dma_start(out=SMDBG[:, g * G:(g + 1) * G],
                                              in_=smg)
                        nc.vector.tensor_reduce(negg[:, :], lgg,
                                                mybir.AxisListType.X, AOP.max,
                                                negate=True)
                        for ii in range(G):
                            et = te2.tile([128, NCLS], f32, tag="et")
                            nc.scalar.activation(et, lgg[:, ii, :], ACT.Exp,
                                                 bias=negg[:, ii:ii + 1],
                                                 accum_out=smg[:, ii:ii + 1])
                        lng = te2.tile([128, G], f32, tag="lng")
                        nc.scalar.activation(lng, smg, ACT.Ln)
                        shg = te2.tile([128, G], f32, tag="shg")
                        nc.vector.tensor_tensor(shg, negg, lng, AOP.subtract)
                        for ii in range(G):
                            tt = g * G + ii
                            ot = te2.tile([128, NCLS], f32, tag="ot")
                            nc.vector.tensor_scalar(ot, lgg[:, ii, :],
                                                    shg[:, ii:ii + 1], None,
                                                    AOP.add)
                            nc.sync.dma_start(
                                out=OUT[tt * 128:(tt + 1) * 128, :], in_=ot)

            spmm_layer(HF, HID, its[1], epi2_tile, epi2_group,
                       (msg2, vp2, psb2))

    nc.compile()
    return nc


_NC_CACHE = {}


def _get_nc(cfg):
    key = (cfg.NT, cfg.KSEG, cfg.SLABC, cfg.NZBIAS)
    if key not in _NC_CACHE:
        _NC_CACHE[key] = _build(cfg, nzbias=cfg.NZBIAS)
    return _NC_CACHE[key]


# ------------------------------------------------------------------ main ---
def kernel(x, edge_row, edge_col, edge_val, W1, b1, W2, b2, Wc, bc,
           _run_kwargs=None):
    from concourse.bass_utils import run_bass_kernel_spmd

    cfg = CFG
    x = np.asarray(x, dtype=np.float32)
    edge_row = np.asarray(edge_row, dtype=np.int64)
    edge_col = np.asarray(edge_col, dtype=np.int64)
    edge_val = np.asarray(edge_val, dtype=np.float32)
    W1 = np.asarray(W1, dtype=np.float32)
    W2 = np.asarray(W2, dtype=np.float32)
    Wc = np.asarray(Wc, dtype=np.float32)
    b1 = np.asarray(b1, dtype=np.float32)
    b2 = np.asarray(b2, dtype=np.float32)
    bc = np.asarray(bc, dtype=np.float32)

    cfg.NZBIAS = bool(np.any(b1) or np.any(b2) or np.any(bc))
    slot_of = _assign_slots(cfg, edge_row, edge_col)
    try:
        idx_all, ldst_all, val_all, deg_all = _plan(
            cfg, edge_row, edge_col, edge_val, slot_of)
    except ValueError:
        cfg.KSEG += 1
        idx_all, ldst_all, val_all, deg_all = _plan(
            cfg, edge_row, edge_col, edge_val, slot_of)

    xg = np.zeros((cfg.NPAD, cfg.IN_DIM), dtype=ml_dtypes.bfloat16)
    xg[slot_of] = x.astype(ml_dtypes.bfloat16)

    w1h = W1.astype(ml_dtypes.bfloat16)
    w2c = (W2 @ Wc).astype(ml_dtypes.bfloat16)
    bcomb = b2 @ Wc
    iota = np.tile(np.arange(128, dtype=np.float32), (128, 1)).astype(
        ml_dtypes.bfloat16)
    b1r = np.tile(b1, (128, 1)).astype(np.float32)
    bcombr = np.tile(bcomb, (128, 1)).astype(np.float32)
    bcr = np.tile(bc, (128, 1)).astype(np.float32)

    nc = _get_nc(cfg)
    in_maps = []
    for c in range(cfg.M):
        in_maps.append({
            "xg": xg, "idx": idx_all[c], "ldst": ldst_all[c],
            "val": val_all[c], "deg": deg_all[c], "w1": w1h, "w2c": w2c,
            "b1r": b1r, "bcombr": bcombr, "bcr": bcr, "iota": iota,
        })
    kw = dict(_run_kwargs or {})
    res = run_bass_kernel_spmd(nc, in_maps, core_ids=list(range(cfg.M)), **kw)
    shard = np.concatenate(
        [res.results[c]["out"] for c in range(cfg.M)], axis=0)  # [NPAD, NCLS]
    out = shard[slot_of]
    kernel.last_results = res
    return out.astype(np.float32)


# revision 23
# speedup vs baseline: 1.7072x; 1.0807x over previous
"""GCN node classifier (2x spmm + classifier + log_softmax) on 8 trn2 cores.

Strategy: destination-node 1D sharding with spmm linearity.
  spmm(A, x@W1 + b1) = (A x)@W1 + deg * b1^T      (deg = rowsum of A)
  spmm(A, h@W2 + b2)@Wc = (A h)@(W2 Wc) + deg * (b2 Wc)^T
so the gather tables are the RAW node features (x bf16 for layer 1,
relu-h bf16 for layer 2) — no dense pre-pass over all nodes, and the
layer weights are applied per dst tile after aggregation.

Each core owns 12,800 dst slots (100 tiles x 128 lanes). Host assigns
nodes to slots with a greedy 4-d balancer so that every (src-quarter,
dst-tile) edge bucket fits in KSEG=4 chunks of 128 edges (the int16
gather index forces 4 quarter views of the 102,400-row table). Per-edge
source rows are fetched with GPSIMD dma_gather (256B rows); the
segment-sum is a tensor-engine matmul against per-chunk scatter
matrices V[e, dst_lane] = edge_val[e] built on DVE with
(iota == ldst) * val, accumulated transposed (psT = Xg^T V) so the
per-tile epilogue can feed psT straight back as lhsT for the weight
matmul. log-softmax is fused per tile. Between layers the per-shard
relu-h table is AllGather'ed into a Shared DRAM tensor.
"""

import numpy as np
import ml_dtypes

from contextlib import ExitStack


# ---------------------------------------------------------------- config ---
class Cfg:
    M = 8                 # cores
    N_NODES = 100000
    N_EDGES = 1600000
    IN_DIM = 128
    HID = 64
    NCLS = 40
    NT = 100              # dst tiles per core (128 lanes each)
    KSEG = 4              # chunks (of 128 edges) per (quarter, tile) segment
    SLABC = 10            # chunks per gather slab
    SINGLE_PACKET = False  # multi-packet gathers (single-packet hangs >~1K idxs)
    NQUEUES = 4           # spread gathers over all 4 SWDGE queues
    MSGBUFS = 28
    IDXBUFS = 1
    GE = 10               # tiles per epilogue-matmul batch
    POOLV = 2             # of 16 chunks/tile, how many V-builds go to Pool
    LNG = 20              # tiles per deferred-Ln group
    NZBIAS = False        # set per-input: any of b1/b2/bc nonzero

    @property
    def PADSHARD(self):
        return self.NT * 128

    @property
    def NPAD(self):
        return self.PADSHARD * self.M

    @property
    def QROWS(self):
        return self.NPAD // 4

    @property
    def SEG(self):
        return self.KSEG * 128

    @property
    def CQ(self):
        return self.NT * self.KSEG          # chunks per quarter

    @property
    def NSLAB(self):
        assert self.CQ % self.SLABC == 0
        return self.CQ // self.SLABC        # gather slabs per quarter

    @property
    def CHUNKS(self):
        return 4 * self.CQ


CFG = Cfg()


# ------------------------------------------------------------- host plan ---
def _assign_slots(cfg, edge_row, edge_col):
    """Assign nodes to table slots so every (src-quarter, dst-tile) edge
    bucket holds <= KSEG*128 edges. Returns slot_of[node] -> [0, NPAD).

    Nodes are first split into 4 fixed quarter groups (so each node's
    src-quarter is pinned), then greedily packed into the 2*NT tiles of
    their own quarter balancing the 4-vector of per-src-quarter in-edge
    counts.
    """
    N, NPAD, QROWS, NT, M = cfg.N_NODES, cfg.NPAD, cfg.QROWS, cfg.NT, cfg.M
    TPQ = QROWS // 128                       # tiles per quarter (2 cores)
    rng = np.random.default_rng(12345)
    order = rng.permutation(N)
    qgrp = np.empty(N, dtype=np.int64)       # node -> quarter group
    npq = N // 4
    for q in range(4):
        qgrp[order[q * npq:(q + 1) * npq]] = q
    qgrp[order[4 * npq:]] = 3

    # per-node in-edge count by source quarter
    cnt = np.zeros((N, 4), dtype=np.int64)
    np.add.at(cnt, (edge_row, qgrp[edge_col]), 1)

    slot_of = np.empty(N, dtype=np.int64)
    for q in range(4):
        nodes = np.where(qgrp == q)[0]
        c = cnt[nodes].astype(np.float32)            # [nq, 4]
        tot = c.sum(1)
        o = np.argsort(-tot, kind="stable")
        nodes, c = nodes[o], c[o]
        loads = np.zeros((TPQ, 4), dtype=np.float32)
        fill = np.zeros(TPQ, dtype=np.int64)
        pos = np.empty(nodes.size, dtype=np.int64)
        for i in range(nodes.size):
            cand = np.max(loads + c[i], axis=1) + (fill >= 128) * 1e9
            b = int(np.argmin(cand))
            loads[b] += c[i]
            pos[i] = b * 128 + fill[b]
            fill[b] += 1
        slot_of[nodes] = q * QROWS + pos
    return slot_of


def _plan(cfg, edge_row, edge_col, edge_val, slot_of):
    """Bucket/sort/pad edges per core. Returns per-core arrays:
    idx16 [128, 4*CQ*128/16] int16, ldstT/valT [128, CHUNKS] bf16,
    plus degs [128, NT] f32 per core.
    """
    M, NT, KSEG, SEG, CQ, QROWS = cfg.M, cfg.NT, cfg.KSEG, cfg.SEG, cfg.CQ, cfg.QROWS
    PADSHARD = cfg.PADSHARD

    src_slot = slot_of[edge_col]
    dst_slot = slot_of[edge_row]
    q_of = src_slot // QROWS
    i_of = src_slot % QROWS
    core_of = dst_slot // PADSHARD
    dloc = dst_slot % PADSHARD
    t_of = dloc // 128
    l_of = dloc % 128

    deg = np.zeros(cfg.NPAD, dtype=np.float64)
    np.add.at(deg, dst_slot, edge_val.astype(np.float64))

    L = 4 * CQ * 128
    idx_all, ldst_all, val_all, deg_all = [], [], [], []
    for c in range(M):
        sel = core_of == c
        segid = q_of[sel] * NT + t_of[sel]
        order = np.argsort(segid, kind="stable")
        sid = segid[order]
        idx_s = i_of[sel][order]
        l_s = l_of[sel][order]
        v_s = edge_val[sel][order]

        counts = np.bincount(sid, minlength=4 * NT)
        if counts.max() > SEG:
            raise ValueError(f"segment overflow: {counts.max()} > {SEG}")
        starts = np.arange(4 * NT) * SEG
        pos = starts[sid] + (np.arange(sid.size)
                             - np.concatenate(([0], np.cumsum(counts)))[sid])

        idx = np.zeros(L, dtype=np.int16)
        ldst = np.zeros(L, dtype=np.float32)
        val = np.zeros(L, dtype=np.float32)
        idx[pos] = idx_s.astype(np.int16)
        ldst[pos] = l_s.astype(np.float32)
        val[pos] = v_s.astype(np.float32)

        # wrap indices: idx i -> [i%16, i//16], replicated on all 8 q7 cores
        idxw = np.tile(idx.reshape(-1, 16).T, (8, 1)).copy()
        ldstT = np.ascontiguousarray(ldst.reshape(-1, 128).T)
        valT = np.ascontiguousarray(val.reshape(-1, 128).T)
        degs = np.ascontiguousarray(
            deg[c * PADSHARD:(c + 1) * PADSHARD].reshape(NT, 128).T
        ).astype(np.float32)
        idx_all.append(idxw)
        ldst_all.append(ldstT)
        val_all.append(valT)
        deg_all.append(degs)
    return idx_all, ldst_all, val_all, deg_all


# --------------------------------------------------------- device program ---
def _build(cfg, timing=False, nzbias=False):
    import os
    from concourse import bacc, tile
    import concourse.mybir as mybir
    kdbg = bool(os.environ.get("KDBG"))

    f32 = mybir.dt.float32
    bf16 = mybir.dt.bfloat16
    i16 = mybir.dt.int16
    AOP = mybir.AluOpType
    ACT = mybir.ActivationFunctionType

    nc = bacc.Bacc("TRN2", target_bir_lowering=False, debug=False,
                   num_devices=1 if timing else cfg.M,
                   dynamic_dma_scratch_size=16384,
                   num_swdge_queues=cfg.NQUEUES)

    NPAD, QROWS, NT, KSEG, CQ, SLABC, NSLAB = (
        cfg.NPAD, cfg.QROWS, cfg.NT, cfg.KSEG, cfg.CQ, cfg.SLABC, cfg.NSLAB)
    CHUNKS, HID, NCLS, IN_DIM = cfg.CHUNKS, cfg.HID, cfg.NCLS, cfg.IN_DIM
    LQ16 = CQ * 128 // 16              # idx columns per quarter
    SLAB16 = SLABC * 128 // 16         # idx columns per slab

    # -------- I/O
    XG = nc.dram_tensor("xg", [NPAD, IN_DIM], bf16, kind="ExternalInput")
    IDX = nc.dram_tensor("idx", [128, 4 * LQ16], i16, kind="ExternalInput")
    LDST = nc.dram_tensor("ldst", [128, CHUNKS], f32, kind="ExternalInput")
    VAL = nc.dram_tensor("val", [128, CHUNKS], f32, kind="ExternalInput")
    DEG = nc.dram_tensor("deg", [128, NT], f32, kind="ExternalInput")
    W1 = nc.dram_tensor("w1", [IN_DIM, HID], bf16, kind="ExternalInput")
    W2C = nc.dram_tensor("w2c", [HID, NCLS], bf16, kind="ExternalInput")
    B1R = nc.dram_tensor("b1r", [128, HID], f32, kind="ExternalInput")
    BCOMBR = nc.dram_tensor("bcombr", [128, NCLS], f32, kind="ExternalInput")
    BCR = nc.dram_tensor("bcr", [128, NCLS], f32, kind="ExternalInput")
    IOTA = nc.dram_tensor("iota", [128, 128], bf16, kind="ExternalInput")
    OUT = nc.dram_tensor("out", [cfg.PADSHARD, NCLS], f32, kind="ExternalOutput")
    HDBG = (nc.dram_tensor("hdbg", [cfg.PADSHARD, HID], bf16,
                           kind="ExternalOutput") if kdbg else None)
    LDBG = (nc.dram_tensor("ldbg", [128, NT * NCLS], f32,
                           kind="ExternalOutput") if kdbg else None)
    SMDBG = (nc.dram_tensor("smdbg", [128, NT], f32,
                            kind="ExternalOutput") if kdbg else None)

    # -------- internal DRAM
    HS = nc.dram_tensor("hshard", [cfg.PADSHARD, 128], bf16)    # cols 64+: junk
    HF = nc.dram_tensor("hfull", [NPAD, 128], bf16, addr_space="Shared")

    with tile.TileContext(nc) as tc, ExitStack() as top:
        cpool = top.enter_context(tc.tile_pool(name="consts", bufs=1))
        w1s = cpool.tile([IN_DIM, HID], bf16)
        nc.sync.dma_start(out=w1s, in_=W1[:, :])
        w2cs = cpool.tile([HID, NCLS], bf16)
        nc.sync.dma_start(out=w2cs, in_=W2C[:, :])
        b1rs = cpool.tile([128, HID], f32)
        nc.sync.dma_start(out=b1rs, in_=B1R[:, :])
        bcombs = cpool.tile([128, NCLS], f32)
        nc.sync.dma_start(out=bcombs, in_=BCOMBR[:, :])
        bcrs = cpool.tile([128, NCLS], f32)
        nc.sync.dma_start(out=bcrs, in_=BCR[:, :])
        iot = cpool.tile([128, 128], bf16)
        nc.sync.dma_start(out=iot, in_=IOTA[:, :])
        degs = cpool.tile([128, NT], f32)
        nc.sync.dma_start(out=degs, in_=DEG[:, :])

        edg = top.enter_context(tc.tile_pool(name="edg", bufs=1))
        ldsts = edg.tile([128, CHUNKS], f32)
        nc.sync.dma_start(out=ldsts, in_=LDST[:, :])
        vals = edg.tile([128, CHUNKS], f32)
        nc.sync.dma_start(out=vals, in_=VAL[:, :])

        # idx tiles for BOTH layers, loaded up front
        idxp = top.enter_context(tc.tile_pool(name="idxp", bufs=1))
        its = []
        for li in range(2):
            row = []
            for q in range(4):
                it = idxp.tile([128, LQ16], i16, tag=f"idx{li}_{q}",
                               name=f"idx{li}_{q}")
                nc.sync.dma_start(out=it, in_=IDX[:, q * LQ16:(q + 1) * LQ16])
                row.append(it)
            its.append(row)

        # ============ spmm layer runner: per-tile single psum group across
        # all 4 quarters, accumulating transposed (psT = Xg^T V); epilogue
        # split into a per-tile part (cast) and a batched per-GE-tiles part
        # (weight matmuls etc) to keep the PE stream free of cross-engine
        # round trips.
        def spmm_layer(tab, width, lits, epi_tile, epi_group, pools):
            msg, vp, psb = pools
            GEB = cfg.GE
            slabs = [[None] * NSLAB for _ in range(4)]

            def ensure_slab(q, s):
                if slabs[q][s] is None:
                    mt = msg.tile([128, SLABC, 128], bf16)
                    nc.gpsimd.dma_gather(
                        mt, tab[q * QROWS:(q + 1) * QROWS, :],
                        lits[q][:, s * SLAB16:(s + 1) * SLAB16],
                        num_idxs=SLABC * 128, num_idxs_reg=SLABC * 128,
                        elem_size=128, elem_step=128,
                        single_packet=cfg.SINGLE_PACKET,
                        queue_num=(q * NSLAB + s) % cfg.NQUEUES)
                    slabs[q][s] = mt
                return slabs[q][s]

            for t in range(NT):
                ps = psb.tile([width, 128], f32)
                for q in range(4):
                    j0 = t * KSEG
                    vt = vp.tile([128, KSEG, 128], bf16)
                    for k in range(KSEG):
                        gj = q * CQ + j0 + k             # global chunk
                        veng = (nc.gpsimd
                                if q * KSEG + k >= 16 - cfg.POOLV
                                else nc.vector)
                        veng.tensor_scalar(
                            vt[:, k, :], iot, ldsts[:, gj:gj + 1],
                            vals[:, gj:gj + 1], AOP.is_equal, AOP.mult)
                    for k in range(KSEG):
                        j = j0 + k                       # chunk in quarter
                        mt = ensure_slab(q, j // SLABC)
                        nc.tensor.matmul(ps, lhsT=mt[:, j % SLABC, 0:width],
                                         rhs=vt[:, k, :],
                                         start=(q == 0 and k == 0),
                                         stop=(q == 3 and k == KSEG - 1))
                epi_tile(t, ps)
                if t % GEB == GEB - 1:
                    epi_group(t - GEB + 1, GEB)

        # ================= layer 1: h = relu((A x)@W1 + deg*b1^T), store bf16
        with tc.tile_pool(name="msg", bufs=cfg.MSGBUFS) as msg, \
             tc.tile_pool(name="vp", bufs=8) as vp, \
             tc.tile_pool(name="psb", bufs=3, space="PSUM") as psb, \
             tc.tile_pool(name="tc1", bufs=cfg.GE + 2) as tp1, \
             tc.tile_pool(name="tc2", bufs=3) as tp2, \
             tc.tile_pool(name="pse", bufs=3, space="PSUM") as pse:
            pss1 = {}

            def epi1_tile(t, ps):
                pss = tp1.tile([IN_DIM, 128], bf16, tag="pss", name="pss")
                nc.scalar.activation(pss, ps, ACT.Copy)
                pss1[t] = pss

            def epi1_group(t0, n):
                for t in range(t0, t0 + n):
                    ph = pse.tile([128, HID], f32)
                    nc.tensor.matmul(ph, lhsT=pss1.pop(t), rhs=w1s,
                                     start=True, stop=True)
                    ht = tp2.tile([128, HID], bf16, tag="ht")
                    if nzbias:
                        tb = tp2.tile([128, HID], f32, tag="tb")
                        nc.vector.tensor_scalar(tb, b1rs, degs[:, t:t + 1],
                                                None, AOP.mult)
                        hsum = tp2.tile([128, HID], f32, tag="hsum")
                        nc.vector.tensor_tensor(hsum, ph, tb, AOP.add)
                        nc.scalar.activation(ht, hsum, ACT.Relu)
                    else:
                        nc.scalar.activation(ht, ph, ACT.Relu)
                    nc.sync.dma_start(out=HS[t * 128:(t + 1) * 128, 0:HID],
                                      in_=ht)

            spmm_layer(XG, IN_DIM, its[0], epi1_tile, epi1_group,
                       (msg, vp, psb))
            if kdbg:
                nc.sync.dma_start(out=HDBG[:, :], in_=HS[:, 0:HID])
            if not timing:
                nc.gpsimd.collective_compute(
                    "AllGather", mybir.AluOpType.bypass,
                    replica_groups=[list(range(cfg.M))],
                    ins=[HS[:, :]], outs=[HF[:, :]])

        # ================= layer 2 + fused classifier/log_softmax
        with tc.tile_pool(name="msg2", bufs=cfg.MSGBUFS) as msg2, \
             tc.tile_pool(name="vp2", bufs=8) as vp2, \
             tc.tile_pool(name="psb2", bufs=3, space="PSUM") as psb2, \
             tc.tile_pool(name="te1", bufs=cfg.GE + 2) as te1, \
             tc.tile_pool(name="te2", bufs=3) as te2, \
             tc.tile_pool(name="te3", bufs=2) as te3, \
             tc.tile_pool(name="psf", bufs=3, space="PSUM") as psf:
            G = cfg.LNG
            assert NT % G == 0 and G % cfg.GE == 0
            pss2 = {}
            state = {}

            def epi2_tile(t, ps):
                pss = te1.tile([HID, 128], bf16, tag="pss", name="pss")
                nc.scalar.activation(pss, ps, ACT.Copy)
                pss2[t] = pss

            def epi2_group(t0, n):
                for t in range(t0, t0 + n):
                    g, i = t // G, t % G
                    if i == 0:
                        state["lgg"] = te3.tile([128, G, NCLS], f32,
                                                tag="lgg", name="lgg")
                        state["negg"] = te3.tile([128, G], f32,
                                                 tag="negg", name="negg")
                        state["smg"] = te3.tile([128, G], f32,
                                                tag="smg", name="smg")
                    lgg, negg, smg = state["lgg"], state["negg"], state["smg"]
                    psl = psf.tile([128, NCLS], f32)
                    nc.tensor.matmul(psl, lhsT=pss2.pop(t), rhs=w2cs,
                                     start=True, stop=True)
                    if nzbias:
                        tb = te2.tile([128, NCLS], f32, tag="tb")
                        nc.vector.tensor_scalar(tb, bcombs, degs[:, t:t + 1],
                                                None, AOP.mult)
                        lg0 = te2.tile([128, NCLS], f32, tag="lg0")
                        nc.vector.tensor_tensor(lg0, psl, tb, AOP.add)
                        nc.gpsimd.tensor_tensor(lgg[:, i, :], lg0, bcrs,
                                                AOP.add)
                    else:
                        nc.scalar.activation(lgg[:, i, :], psl, ACT.Copy)
                    if i == G - 1:
                        if kdbg:
                            nc.sync.dma_start(
                                out=LDBG[:, g * G * NCLS:(g + 1) * G * NCLS],
                                in_=lgg.rearrange("p a b -> p (a b)"))
                            nc.sync.dma_start(out=SMDBG[:, g * G:(g + 1) * G],
                                              in_=smg)
                        nc.vector.tensor_reduce(negg[:, :], lgg,
                                                mybir.AxisListType.X, AOP.max,
                                                negate=True)
                        for ii in range(G):
                            et = te2.tile([128, NCLS], f32, tag="et")
                            nc.scalar.activation(et, lgg[:, ii, :], ACT.Exp,
                                                 bias=negg[:, ii:ii + 1],
                                                 accum_out=smg[:, ii:ii + 1])
                        lng = te2.tile([128, G], f32, tag="lng")
                        nc.scalar.activation(lng, smg, ACT.Ln)
                        shg = te2.tile([128, G], f32, tag="shg")
                        nc.vector.tensor_tensor(shg, negg, lng, AOP.subtract)
                        for ii in range(G):
                            tt = g * G + ii
                            ot = te2.tile([128, NCLS], f32, tag="ot")
                            nc.vector.tensor_scalar(ot, lgg[:, ii, :],
                                                    shg[:, ii:ii + 1], None,
                                                    AOP.add)
                            nc.sync.dma_start(
                                out=OUT[tt * 128:(tt + 1) * 128, :], in_=ot)

            spmm_layer(HF, HID, its[1], epi2_tile, epi2_group,
                       (msg2, vp2, psb2))

    nc.compile()
    return nc


_NC_CACHE = {}


def _get_nc(cfg):
    key = (cfg.NT, cfg.KSEG, cfg.SLABC, cfg.NZBIAS)
    if key not in _NC_CACHE:
        _NC_CACHE[key] = _build(cfg, nzbias=cfg.NZBIAS)
    return _NC_CACHE[key]


# ------------------------------------------------------------------ main ---
def kernel(x, edge_row, edge_col, edge_val, W1, b1, W2, b2, Wc, bc,
           _run_kwargs=None):
    from concourse.bass_utils import run_bass_kernel_spmd

    cfg = CFG
    x = np.asarray(x, dtype=np.float32)
    edge_row = np.asarray(edge_row, dtype=np.int64)
    edge_col = np.asarray(edge_col, dtype=np.int64)
    edge_val = np.asarray(edge_val, dtype=np.float32)
    W1 = np.asarray(W1, dtype=np.float32)
    W2 = np.asarray(W2, dtype=np.float32)
    Wc = np.asarray(Wc, dtype=np.float32)
    b1 = np.asarray(b1, dtype=np.float32)
    b2 = np.asarray(b2, dtype=np.float32)
    bc = np.asarray(bc, dtype=np.float32)

    cfg.NZBIAS = bool(np.any(b1) or np.any(b2) or np.any(bc))
    slot_of = _assign_slots(cfg, edge_row, edge_col)
    try:
        idx_all, ldst_all, val_all, deg_all = _plan(
            cfg, edge_row, edge_col, edge_val, slot_of)
    except ValueError:
        cfg.KSEG += 1
        idx_all, ldst_all, val_all, deg_all = _plan(
            cfg, edge_row, edge_col, edge_val, slot_of)

    xg = np.zeros((cfg.NPAD, cfg.IN_DIM), dtype=ml_dtypes.bfloat16)
    xg[slot_of] = x.astype(ml_dtypes.bfloat16)

    w1h = W1.astype(ml_dtypes.bfloat16)
    w2c = (W2 @ Wc).astype(ml_dtypes.bfloat16)
    bcomb = b2 @ Wc
    iota = np.tile(np.arange(128, dtype=np.float32), (128, 1)).astype(
        ml_dtypes.bfloat16)
    b1r = np.tile(b1, (128, 1)).astype(np.float32)
    bcombr = np.tile(bcomb, (128, 1)).astype(np.float32)
    bcr = np.tile(bc, (128, 1)).astype(np.float32)

    nc = _get_nc(cfg)
    in_maps = []
    for c in range(cfg.M):
        in_maps.append({
            "xg": xg, "idx": idx_all[c], "ldst": ldst_all[c],
            "val": val_all[c], "deg": deg_all[c], "w1": w1h, "w2c": w2c,
            "b1r": b1r, "bcombr": bcombr, "bcr": bcr, "iota": iota,
        })
    kw = dict(_run_kwargs or {})
    res = run_bass_kernel_spmd(nc, in_maps, core_ids=list(range(cfg.M)), **kw)
    shard = np.concatenate(
        [res.results[c]["out"] for c in range(cfg.M)], axis=0)  # [NPAD, NCLS]
    out = shard[slot_of]
    kernel.last_results = res
    return out.astype(np.float32)


# revision 24
# speedup vs baseline: 1.7767x; 1.0407x over previous
"""GCN node classifier (2x spmm + classifier + log_softmax) on 8 trn2 cores.

Strategy: destination-node 1D sharding with spmm linearity.
  spmm(A, x@W1 + b1) = (A x)@W1 + deg * b1^T      (deg = rowsum of A)
  spmm(A, h@W2 + b2)@Wc = (A h)@(W2 Wc) + deg * (b2 Wc)^T
so the gather tables are the RAW node features (x bf16 for layer 1,
relu-h bf16 for layer 2) — no dense pre-pass over all nodes, and the
layer weights are applied per dst tile after aggregation.

Each core owns 12,800 dst slots (100 tiles x 128 lanes). Host assigns
nodes to slots with a greedy 4-d balancer so that every (src-quarter,
dst-tile) edge bucket fits in KSEG=4 chunks of 128 edges (the int16
gather index forces 4 quarter views of the 102,400-row table). Per-edge
source rows are fetched with GPSIMD dma_gather (256B rows); the
segment-sum is a tensor-engine matmul against per-chunk scatter
matrices V[e, dst_lane] = edge_val[e] built on DVE with
(iota == ldst) * val, accumulated transposed (psT = Xg^T V) so the
per-tile epilogue can feed psT straight back as lhsT for the weight
matmul. log-softmax is fused per tile. Between layers the per-shard
relu-h table is AllGather'ed into a Shared DRAM tensor.
"""

import numpy as np
import ml_dtypes

from contextlib import ExitStack


# ---------------------------------------------------------------- config ---
class Cfg:
    M = 8                 # cores
    N_NODES = 100000
    N_EDGES = 1600000
    IN_DIM = 128
    HID = 64
    NCLS = 40
    NT = 100              # dst tiles per core (128 lanes each)
    KSEG = 4              # chunks (of 128 edges) per (quarter, tile) segment
    SLABC = 10            # chunks per gather slab
    SINGLE_PACKET = False  # multi-packet gathers (single-packet hangs >~1K idxs)
    NQUEUES = 4           # spread gathers over all 4 SWDGE queues
    MSGBUFS = 28
    IDXBUFS = 1
    GE = 5                # tiles per epilogue-matmul batch
    POOLV = 2             # of 16 chunks/tile, how many V-builds go to Pool
    LNG = 5               # tiles per deferred-Ln group
    NZBIAS = False        # set per-input: any of b1/b2/bc nonzero

    @property
    def PADSHARD(self):
        return self.NT * 128

    @property
    def NPAD(self):
        return self.PADSHARD * self.M

    @property
    def QROWS(self):
        return self.NPAD // 4

    @property
    def SEG(self):
        return self.KSEG * 128

    @property
    def CQ(self):
        return self.NT * self.KSEG          # chunks per quarter

    @property
    def NSLAB(self):
        assert self.CQ % self.SLABC == 0
        return self.CQ // self.SLABC        # gather slabs per quarter

    @property
    def CHUNKS(self):
        return 4 * self.CQ


CFG = Cfg()


# ------------------------------------------------------------- host plan ---
def _assign_slots(cfg, edge_row, edge_col):
    """Assign nodes to table slots so every (src-quarter, dst-tile) edge
    bucket holds <= KSEG*128 edges. Returns slot_of[node] -> [0, NPAD).

    Nodes are first split into 4 fixed quarter groups (so each node's
    src-quarter is pinned), then greedily packed into the 2*NT tiles of
    their own quarter balancing the 4-vector of per-src-quarter in-edge
    counts.
    """
    N, NPAD, QROWS, NT, M = cfg.N_NODES, cfg.NPAD, cfg.QROWS, cfg.NT, cfg.M
    TPQ = QROWS // 128                       # tiles per quarter (2 cores)
    rng = np.random.default_rng(12345)
    order = rng.permutation(N)
    qgrp = np.empty(N, dtype=np.int64)       # node -> quarter group
    npq = N // 4
    for q in range(4):
        qgrp[order[q * npq:(q + 1) * npq]] = q
    qgrp[order[4 * npq:]] = 3

    # per-node in-edge count by source quarter
    cnt = np.zeros((N, 4), dtype=np.int64)
    np.add.at(cnt, (edge_row, qgrp[edge_col]), 1)

    slot_of = np.empty(N, dtype=np.int64)
    for q in range(4):
        nodes = np.where(qgrp == q)[0]
        c = cnt[nodes].astype(np.float32)            # [nq, 4]
        tot = c.sum(1)
        o = np.argsort(-tot, kind="stable")
        nodes, c = nodes[o], c[o]
        loads = np.zeros((TPQ, 4), dtype=np.float32)
        fill = np.zeros(TPQ, dtype=np.int64)
        pos = np.empty(nodes.size, dtype=np.int64)
        for i in range(nodes.size):
            cand = np.max(loads + c[i], axis=1) + (fill >= 128) * 1e9
            b = int(np.argmin(cand))
            loads[b] += c[i]
            pos[i] = b * 128 + fill[b]
            fill[b] += 1
        slot_of[nodes] = q * QROWS + pos
    return slot_of


def _plan(cfg, edge_row, edge_col, edge_val, slot_of):
    """Bucket/sort/pad edges per core. Returns per-core arrays:
    idx16 [128, 4*CQ*128/16] int16, ldstT/valT [128, CHUNKS] bf16,
    plus degs [128, NT] f32 per core.
    """
    M, NT, KSEG, SEG, CQ, QROWS = cfg.M, cfg.NT, cfg.KSEG, cfg.SEG, cfg.CQ, cfg.QROWS
    PADSHARD = cfg.PADSHARD

    src_slot = slot_of[edge_col]
    dst_slot = slot_of[edge_row]
    q_of = src_slot // QROWS
    i_of = src_slot % QROWS
    core_of = dst_slot // PADSHARD
    dloc = dst_slot % PADSHARD
    t_of = dloc // 128
    l_of = dloc % 128

    deg = np.zeros(cfg.NPAD, dtype=np.float64)
    np.add.at(deg, dst_slot, edge_val.astype(np.float64))

    L = 4 * CQ * 128
    idx_all, ldst_all, val_all, deg_all = [], [], [], []
    for c in range(M):
        sel = core_of == c
        segid = q_of[sel] * NT + t_of[sel]
        order = np.argsort(segid, kind="stable")
        sid = segid[order]
        idx_s = i_of[sel][order]
        l_s = l_of[sel][order]
        v_s = edge_val[sel][order]

        counts = np.bincount(sid, minlength=4 * NT)
        if counts.max() > SEG:
            raise ValueError(f"segment overflow: {counts.max()} > {SEG}")
        starts = np.arange(4 * NT) * SEG
        pos = starts[sid] + (np.arange(sid.size)
                             - np.concatenate(([0], np.cumsum(counts)))[sid])

        idx = np.zeros(L, dtype=np.int16)
        ldst = np.zeros(L, dtype=np.float32)
        val = np.zeros(L, dtype=np.float32)
        idx[pos] = idx_s.astype(np.int16)
        ldst[pos] = l_s.astype(np.float32)
        val[pos] = v_s.astype(np.float32)

        # wrap indices: idx i -> [i%16, i//16], replicated on all 8 q7 cores
        idxw = np.tile(idx.reshape(-1, 16).T, (8, 1)).copy()
        ldstT = np.ascontiguousarray(ldst.reshape(-1, 128).T)
        valT = np.ascontiguousarray(val.reshape(-1, 128).T)
        degs = np.ascontiguousarray(
            deg[c * PADSHARD:(c + 1) * PADSHARD].reshape(NT, 128).T
        ).astype(np.float32)
        idx_all.append(idxw)
        ldst_all.append(ldstT)
        val_all.append(valT)
        deg_all.append(degs)
    return idx_all, ldst_all, val_all, deg_all


# --------------------------------------------------------- device program ---
def _build(cfg, timing=False, nzbias=False):
    import os
    from concourse import bacc, tile
    import concourse.mybir as mybir
    kdbg = bool(os.environ.get("KDBG"))

    f32 = mybir.dt.float32
    bf16 = mybir.dt.bfloat16
    i16 = mybir.dt.int16
    AOP = mybir.AluOpType
    ACT = mybir.ActivationFunctionType

    nc = bacc.Bacc("TRN2", target_bir_lowering=False, debug=False,
                   num_devices=1 if timing else cfg.M,
                   dynamic_dma_scratch_size=16384,
                   num_swdge_queues=cfg.NQUEUES)

    NPAD, QROWS, NT, KSEG, CQ, SLABC, NSLAB = (
        cfg.NPAD, cfg.QROWS, cfg.NT, cfg.KSEG, cfg.CQ, cfg.SLABC, cfg.NSLAB)
    CHUNKS, HID, NCLS, IN_DIM = cfg.CHUNKS, cfg.HID, cfg.NCLS, cfg.IN_DIM
    LQ16 = CQ * 128 // 16              # idx columns per quarter
    SLAB16 = SLABC * 128 // 16         # idx columns per slab

    # -------- I/O
    XG = nc.dram_tensor("xg", [NPAD, IN_DIM], bf16, kind="ExternalInput")
    IDX = nc.dram_tensor("idx", [128, 4 * LQ16], i16, kind="ExternalInput")
    LDST = nc.dram_tensor("ldst", [128, CHUNKS], f32, kind="ExternalInput")
    VAL = nc.dram_tensor("val", [128, CHUNKS], f32, kind="ExternalInput")
    DEG = nc.dram_tensor("deg", [128, NT], f32, kind="ExternalInput")
    W1 = nc.dram_tensor("w1", [IN_DIM, HID], bf16, kind="ExternalInput")
    W2C = nc.dram_tensor("w2c", [HID, NCLS], bf16, kind="ExternalInput")
    B1R = nc.dram_tensor("b1r", [128, HID], f32, kind="ExternalInput")
    BCOMBR = nc.dram_tensor("bcombr", [128, NCLS], f32, kind="ExternalInput")
    BCR = nc.dram_tensor("bcr", [128, NCLS], f32, kind="ExternalInput")
    IOTA = nc.dram_tensor("iota", [128, 128], bf16, kind="ExternalInput")
    OUT = nc.dram_tensor("out", [cfg.PADSHARD, NCLS], f32, kind="ExternalOutput")
    HDBG = (nc.dram_tensor("hdbg", [cfg.PADSHARD, HID], bf16,
                           kind="ExternalOutput") if kdbg else None)
    LDBG = (nc.dram_tensor("ldbg", [128, NT * NCLS], f32,
                           kind="ExternalOutput") if kdbg else None)
    SMDBG = (nc.dram_tensor("smdbg", [128, NT], f32,
                            kind="ExternalOutput") if kdbg else None)

    # -------- internal DRAM
    HS = nc.dram_tensor("hshard", [cfg.PADSHARD, 128], bf16)    # cols 64+: junk
    HF = nc.dram_tensor("hfull", [NPAD, 128], bf16, addr_space="Shared")

    with tile.TileContext(nc) as tc, ExitStack() as top:
        cpool = top.enter_context(tc.tile_pool(name="consts", bufs=1))
        w1s = cpool.tile([IN_DIM, HID], bf16)
        nc.sync.dma_start(out=w1s, in_=W1[:, :])
        w2cs = cpool.tile([HID, NCLS], bf16)
        nc.sync.dma_start(out=w2cs, in_=W2C[:, :])
        b1rs = cpool.tile([128, HID], f32)
        nc.sync.dma_start(out=b1rs, in_=B1R[:, :])
        bcombs = cpool.tile([128, NCLS], f32)
        nc.sync.dma_start(out=bcombs, in_=BCOMBR[:, :])
        bcrs = cpool.tile([128, NCLS], f32)
        nc.sync.dma_start(out=bcrs, in_=BCR[:, :])
        iot = cpool.tile([128, 128], bf16)
        nc.sync.dma_start(out=iot, in_=IOTA[:, :])
        degs = cpool.tile([128, NT], f32)
        nc.sync.dma_start(out=degs, in_=DEG[:, :])

        edg = top.enter_context(tc.tile_pool(name="edg", bufs=1))
        ldsts = edg.tile([128, CHUNKS], f32)
        nc.sync.dma_start(out=ldsts, in_=LDST[:, :])
        vals = edg.tile([128, CHUNKS], f32)
        nc.sync.dma_start(out=vals, in_=VAL[:, :])

        # idx tiles for BOTH layers, loaded up front
        idxp = top.enter_context(tc.tile_pool(name="idxp", bufs=1))
        its = []
        for li in range(2):
            row = []
            for q in range(4):
                it = idxp.tile([128, LQ16], i16, tag=f"idx{li}_{q}",
                               name=f"idx{li}_{q}")
                nc.sync.dma_start(out=it, in_=IDX[:, q * LQ16:(q + 1) * LQ16])
                row.append(it)
            its.append(row)

        # ============ spmm layer runner: per-tile single psum group across
        # all 4 quarters, accumulating transposed (psT = Xg^T V); epilogue
        # split into a per-tile part (cast) and a batched per-GE-tiles part
        # (weight matmuls etc) to keep the PE stream free of cross-engine
        # round trips.
        def spmm_layer(tab, width, lits, epi_tile, epi_group, pools):
            msg, vp, psb = pools
            GEB = cfg.GE
            slabs = [[None] * NSLAB for _ in range(4)]

            def ensure_slab(q, s):
                if slabs[q][s] is None:
                    mt = msg.tile([128, SLABC, 128], bf16)
                    nc.gpsimd.dma_gather(
                        mt, tab[q * QROWS:(q + 1) * QROWS, :],
                        lits[q][:, s * SLAB16:(s + 1) * SLAB16],
                        num_idxs=SLABC * 128, num_idxs_reg=SLABC * 128,
                        elem_size=128, elem_step=128,
                        single_packet=cfg.SINGLE_PACKET,
                        queue_num=(q * NSLAB + s) % cfg.NQUEUES)
                    slabs[q][s] = mt
                return slabs[q][s]

            for t in range(NT):
                ps = psb.tile([width, 128], f32)
                for q in range(4):
                    j0 = t * KSEG
                    vt = vp.tile([128, KSEG, 128], bf16)
                    for k in range(KSEG):
                        gj = q * CQ + j0 + k             # global chunk
                        veng = (nc.gpsimd
                                if q * KSEG + k >= 16 - cfg.POOLV
                                else nc.vector)
                        veng.tensor_scalar(
                            vt[:, k, :], iot, ldsts[:, gj:gj + 1],
                            vals[:, gj:gj + 1], AOP.is_equal, AOP.mult)
                    for k in range(KSEG):
                        j = j0 + k                       # chunk in quarter
                        mt = ensure_slab(q, j // SLABC)
                        nc.tensor.matmul(ps, lhsT=mt[:, j % SLABC, 0:width],
                                         rhs=vt[:, k, :],
                                         start=(q == 0 and k == 0),
                                         stop=(q == 3 and k == KSEG - 1))
                epi_tile(t, ps)
                if t % GEB == GEB - 1:
                    epi_group(t - GEB + 1, GEB)

        # ================= layer 1: h = relu((A x)@W1 + deg*b1^T), store bf16
        with tc.tile_pool(name="msg", bufs=cfg.MSGBUFS) as msg, \
             tc.tile_pool(name="vp", bufs=8) as vp, \
             tc.tile_pool(name="psb", bufs=3, space="PSUM") as psb, \
             tc.tile_pool(name="tc1", bufs=cfg.GE + 2) as tp1, \
             tc.tile_pool(name="tc2", bufs=3) as tp2, \
             tc.tile_pool(name="pse", bufs=3, space="PSUM") as pse:
            pss1 = {}

            def epi1_tile(t, ps):
                pss = tp1.tile([IN_DIM, 128], bf16, tag="pss", name="pss")
                nc.scalar.activation(pss, ps, ACT.Copy)
                pss1[t] = pss

            def epi1_group(t0, n):
                for t in range(t0, t0 + n):
                    ph = pse.tile([128, HID], f32)
                    nc.tensor.matmul(ph, lhsT=pss1.pop(t), rhs=w1s,
                                     start=True, stop=True)
                    ht = tp2.tile([128, HID], bf16, tag="ht")
                    if nzbias:
                        tb = tp2.tile([128, HID], f32, tag="tb")
                        nc.vector.tensor_scalar(tb, b1rs, degs[:, t:t + 1],
                                                None, AOP.mult)
                        hsum = tp2.tile([128, HID], f32, tag="hsum")
                        nc.vector.tensor_tensor(hsum, ph, tb, AOP.add)
                        nc.scalar.activation(ht, hsum, ACT.Relu)
                    else:
                        nc.scalar.activation(ht, ph, ACT.Relu)
                    nc.sync.dma_start(out=HS[t * 128:(t + 1) * 128, 0:HID],
                                      in_=ht)

            spmm_layer(XG, IN_DIM, its[0], epi1_tile, epi1_group,
                       (msg, vp, psb))
            if kdbg:
                nc.sync.dma_start(out=HDBG[:, :], in_=HS[:, 0:HID])
            if not timing:
                nc.gpsimd.collective_compute(
                    "AllGather", mybir.AluOpType.bypass,
                    replica_groups=[list(range(cfg.M))],
                    ins=[HS[:, :]], outs=[HF[:, :]])

        # ================= layer 2 + fused classifier/log_softmax
        with tc.tile_pool(name="msg2", bufs=cfg.MSGBUFS) as msg2, \
             tc.tile_pool(name="vp2", bufs=8) as vp2, \
             tc.tile_pool(name="psb2", bufs=3, space="PSUM") as psb2, \
             tc.tile_pool(name="te1", bufs=cfg.GE + 2) as te1, \
             tc.tile_pool(name="te2", bufs=3) as te2, \
             tc.tile_pool(name="te3", bufs=2) as te3, \
             tc.tile_pool(name="psf", bufs=3, space="PSUM") as psf:
            G = cfg.LNG
            assert NT % G == 0 and G % cfg.GE == 0
            pss2 = {}
            state = {}

            def epi2_tile(t, ps):
                pss = te1.tile([HID, 128], bf16, tag="pss", name="pss")
                nc.scalar.activation(pss, ps, ACT.Copy)
                pss2[t] = pss

            def epi2_group(t0, n):
                for t in range(t0, t0 + n):
                    g, i = t // G, t % G
                    if i == 0:
                        state["lgg"] = te3.tile([128, G, NCLS], f32,
                                                tag="lgg", name="lgg")
                        state["negg"] = te3.tile([128, G], f32,
                                                 tag="negg", name="negg")
                        state["smg"] = te3.tile([128, G], f32,
                                                tag="smg", name="smg")
                    lgg, negg, smg = state["lgg"], state["negg"], state["smg"]
                    psl = psf.tile([128, NCLS], f32)
                    nc.tensor.matmul(psl, lhsT=pss2.pop(t), rhs=w2cs,
                                     start=True, stop=True)
                    if nzbias:
                        tb = te2.tile([128, NCLS], f32, tag="tb")
                        nc.vector.tensor_scalar(tb, bcombs, degs[:, t:t + 1],
                                                None, AOP.mult)
                        lg0 = te2.tile([128, NCLS], f32, tag="lg0")
                        nc.vector.tensor_tensor(lg0, psl, tb, AOP.add)
                        nc.gpsimd.tensor_tensor(lgg[:, i, :], lg0, bcrs,
                                                AOP.add)
                    else:
                        nc.scalar.activation(lgg[:, i, :], psl, ACT.Copy)
                    if i == G - 1:
                        if kdbg:
                            nc.sync.dma_start(
                                out=LDBG[:, g * G * NCLS:(g + 1) * G * NCLS],
                                in_=lgg.rearrange("p a b -> p (a b)"))
                            nc.sync.dma_start(out=SMDBG[:, g * G:(g + 1) * G],
                                              in_=smg)
                        nc.vector.tensor_reduce(negg[:, :], lgg,
                                                mybir.AxisListType.X, AOP.max,
                                                negate=True)
                        for ii in range(G):
                            et = te2.tile([128, NCLS], f32, tag="et")
                            nc.scalar.activation(et, lgg[:, ii, :], ACT.Exp,
                                                 bias=negg[:, ii:ii + 1],
                                                 accum_out=smg[:, ii:ii + 1])
                        lng = te2.tile([128, G], f32, tag="lng")
                        nc.scalar.activation(lng, smg, ACT.Ln)
                        shg = te2.tile([128, G], f32, tag="shg")
                        nc.vector.tensor_tensor(shg, negg, lng, AOP.subtract)
                        for ii in range(G):
                            tt = g * G + ii
                            ot = te2.tile([128, NCLS], f32, tag="ot")
                            nc.vector.tensor_scalar(ot, lgg[:, ii, :],
                                                    shg[:, ii:ii + 1], None,
                                                    AOP.add)
                            nc.sync.dma_start(
                                out=OUT[tt * 128:(tt + 1) * 128, :], in_=ot)

            spmm_layer(HF, HID, its[1], epi2_tile, epi2_group,
                       (msg2, vp2, psb2))

    nc.compile()
    return nc


_NC_CACHE = {}


def _get_nc(cfg):
    key = (cfg.NT, cfg.KSEG, cfg.SLABC, cfg.NZBIAS)
    if key not in _NC_CACHE:
        _NC_CACHE[key] = _build(cfg, nzbias=cfg.NZBIAS)
    return _NC_CACHE[key]


# ------------------------------------------------------------------ main ---
def kernel(x, edge_row, edge_col, edge_val, W1, b1, W2, b2, Wc, bc,
           _run_kwargs=None):
    from concourse.bass_utils import run_bass_kernel_spmd

    cfg = CFG
    x = np.asarray(x, dtype=np.float32)
    edge_row = np.asarray(edge_row, dtype=np.int64)
    edge_col = np.asarray(edge_col, dtype=np.int64)
    edge_val = np.asarray(edge_val, dtype=np.float32)
    W1 = np.asarray(W1, dtype=np.float32)
    W2 = np.asarray(W2, dtype=np.float32)
    Wc = np.asarray(Wc, dtype=np.float32)
    b1 = np.asarray(b1, dtype=np.float32)
    b2 = np.asarray(b2, dtype=np.float32)
    bc = np.asarray(bc, dtype=np.float32)

    cfg.NZBIAS = bool(np.any(b1) or np.any(b2) or np.any(bc))
    slot_of = _assign_slots(cfg, edge_row, edge_col)
    try:
        idx_all, ldst_all, val_all, deg_all = _plan(
            cfg, edge_row, edge_col, edge_val, slot_of)
    except ValueError:
        cfg.KSEG += 1
        idx_all, ldst_all, val_all, deg_all = _plan(
            cfg, edge_row, edge_col, edge_val, slot_of)

    xg = np.zeros((cfg.NPAD, cfg.IN_DIM), dtype=ml_dtypes.bfloat16)
    xg[slot_of] = x.astype(ml_dtypes.bfloat16)

    w1h = W1.astype(ml_dtypes.bfloat16)
    w2c = (W2 @ Wc).astype(ml_dtypes.bfloat16)
    bcomb = b2 @ Wc
    iota = np.tile(np.arange(128, dtype=np.float32), (128, 1)).astype(
        ml_dtypes.bfloat16)
    b1r = np.tile(b1, (128, 1)).astype(np.float32)
    bcombr = np.tile(bcomb, (128, 1)).astype(np.float32)
    bcr = np.tile(bc, (128, 1)).astype(np.float32)

    nc = _get_nc(cfg)
    in_maps = []
    for c in range(cfg.M):
        in_maps.append({
            "xg": xg, "idx": idx_all[c], "ldst": ldst_all[c],
            "val": val_all[c], "deg": deg_all[c], "w1": w1h, "w2c": w2c,
            "b1r": b1r, "bcombr": bcombr, "bcr": bcr, "iota": iota,
        })
    kw = dict(_run_kwargs or {})
    res = run_bass_kernel_spmd(nc, in_maps, core_ids=list(range(cfg.M)), **kw)
    shard = np.concatenate(
        [res.results[c]["out"] for c in range(cfg.M)], axis=0)  # [NPAD, NCLS]
    out = shard[slot_of]
    kernel.last_results = res
    return out.astype(np.float32)


# revision 25
# speedup vs baseline: 1.7807x; 1.0023x over previous
"""GCN node classifier (2x spmm + classifier + log_softmax) on 8 trn2 cores.

Strategy: destination-node 1D sharding with spmm linearity.
  spmm(A, x@W1 + b1) = (A x)@W1 + deg * b1^T      (deg = rowsum of A)
  spmm(A, h@W2 + b2)@Wc = (A h)@(W2 Wc) + deg * (b2 Wc)^T
so the gather tables are the RAW node features (x bf16 for layer 1,
relu-h bf16 for layer 2) — no dense pre-pass over all nodes, and the
layer weights are applied per dst tile after aggregation.

Each core owns 12,800 dst slots (100 tiles x 128 lanes). Host assigns
nodes to slots with a greedy 4-d balancer so that every (src-quarter,
dst-tile) edge bucket fits in KSEG=4 chunks of 128 edges (the int16
gather index forces 4 quarter views of the 102,400-row table). Per-edge
source rows are fetched with GPSIMD dma_gather (256B rows); the
segment-sum is a tensor-engine matmul against per-chunk scatter
matrices V[e, dst_lane] = edge_val[e] built on DVE with
(iota == ldst) * val, accumulated transposed (psT = Xg^T V) so the
per-tile epilogue can feed psT straight back as lhsT for the weight
matmul. log-softmax is fused per tile. Between layers the per-shard
relu-h table is AllGather'ed into a Shared DRAM tensor.
"""

import numpy as np
import ml_dtypes

from contextlib import ExitStack


# ---------------------------------------------------------------- config ---
class Cfg:
    M = 8                 # cores
    N_NODES = 100000
    N_EDGES = 1600000
    IN_DIM = 128
    HID = 64
    NCLS = 40
    NT = 100              # dst tiles per core (128 lanes each)
    KSEG = 4              # chunks (of 128 edges) per (quarter, tile) segment
    SLABC = 10            # chunks per gather slab
    SINGLE_PACKET = False  # multi-packet gathers (single-packet hangs >~1K idxs)
    NQUEUES = 4           # spread gathers over all 4 SWDGE queues
    MSGBUFS = 28
    IDXBUFS = 1
    GE = 5                # tiles per epilogue-matmul batch
    POOLV = 1             # of 16 chunks/tile, how many V-builds go to Pool
    LNG = 5               # tiles per deferred-Ln group
    NZBIAS = False        # set per-input: any of b1/b2/bc nonzero

    @property
    def PADSHARD(self):
        return self.NT * 128

    @property
    def NPAD(self):
        return self.PADSHARD * self.M

    @property
    def QROWS(self):
        return self.NPAD // 4

    @property
    def SEG(self):
        return self.KSEG * 128

    @property
    def CQ(self):
        return self.NT * self.KSEG          # chunks per quarter

    @property
    def NSLAB(self):
        assert self.CQ % self.SLABC == 0
        return self.CQ // self.SLABC        # gather slabs per quarter

    @property
    def CHUNKS(self):
        return 4 * self.CQ


CFG = Cfg()


# ------------------------------------------------------------- host plan ---
def _assign_slots(cfg, edge_row, edge_col):
    """Assign nodes to table slots so every (src-quarter, dst-tile) edge
    bucket holds <= KSEG*128 edges. Returns slot_of[node] -> [0, NPAD).

    Nodes are first split into 4 fixed quarter groups (so each node's
    src-quarter is pinned), then greedily packed into the 2*NT tiles of
    their own quarter balancing the 4-vector of per-src-quarter in-edge
    counts.
    """
    N, NPAD, QROWS, NT, M = cfg.N_NODES, cfg.NPAD, cfg.QROWS, cfg.NT, cfg.M
    TPQ = QROWS // 128                       # tiles per quarter (2 cores)
    rng = np.random.default_rng(12345)
    order = rng.permutation(N)
    qgrp = np.empty(N, dtype=np.int64)       # node -> quarter group
    npq = N // 4
    for q in range(4):
        qgrp[order[q * npq:(q + 1) * npq]] = q
    qgrp[order[4 * npq:]] = 3

    # per-node in-edge count by source quarter
    cnt = np.zeros((N, 4), dtype=np.int64)
    np.add.at(cnt, (edge_row, qgrp[edge_col]), 1)

    slot_of = np.empty(N, dtype=np.int64)
    for q in range(4):
        nodes = np.where(qgrp == q)[0]
        c = cnt[nodes].astype(np.float32)            # [nq, 4]
        tot = c.sum(1)
        o = np.argsort(-tot, kind="stable")
        nodes, c = nodes[o], c[o]
        loads = np.zeros((TPQ, 4), dtype=np.float32)
        fill = np.zeros(TPQ, dtype=np.int64)
        pos = np.empty(nodes.size, dtype=np.int64)
        for i in range(nodes.size):
            cand = np.max(loads + c[i], axis=1) + (fill >= 128) * 1e9
            b = int(np.argmin(cand))
            loads[b] += c[i]
            pos[i] = b * 128 + fill[b]
            fill[b] += 1
        slot_of[nodes] = q * QROWS + pos
    return slot_of


def _plan(cfg, edge_row, edge_col, edge_val, slot_of):
    """Bucket/sort/pad edges per core. Returns per-core arrays:
    idx16 [128, 4*CQ*128/16] int16, ldstT/valT [128, CHUNKS] bf16,
    plus degs [128, NT] f32 per core.
    """
    M, NT, KSEG, SEG, CQ, QROWS = cfg.M, cfg.NT, cfg.KSEG, cfg.SEG, cfg.CQ, cfg.QROWS
    PADSHARD = cfg.PADSHARD

    src_slot = slot_of[edge_col]
    dst_slot = slot_of[edge_row]
    q_of = src_slot // QROWS
    i_of = src_slot % QROWS
    core_of = dst_slot // PADSHARD
    dloc = dst_slot % PADSHARD
    t_of = dloc // 128
    l_of = dloc % 128

    deg = np.zeros(cfg.NPAD, dtype=np.float64)
    np.add.at(deg, dst_slot, edge_val.astype(np.float64))

    L = 4 * CQ * 128
    idx_all, ldst_all, val_all, deg_all = [], [], [], []
    for c in range(M):
        sel = core_of == c
        segid = q_of[sel] * NT + t_of[sel]
        order = np.argsort(segid, kind="stable")
        sid = segid[order]
        idx_s = i_of[sel][order]
        l_s = l_of[sel][order]
        v_s = edge_val[sel][order]

        counts = np.bincount(sid, minlength=4 * NT)
        if counts.max() > SEG:
            raise ValueError(f"segment overflow: {counts.max()} > {SEG}")
        starts = np.arange(4 * NT) * SEG
        pos = starts[sid] + (np.arange(sid.size)
                             - np.concatenate(([0], np.cumsum(counts)))[sid])

        idx = np.zeros(L, dtype=np.int16)
        ldst = np.zeros(L, dtype=np.float32)
        val = np.zeros(L, dtype=np.float32)
        idx[pos] = idx_s.astype(np.int16)
        ldst[pos] = l_s.astype(np.float32)
        val[pos] = v_s.astype(np.float32)

        # wrap indices: idx i -> [i%16, i//16], replicated on all 8 q7 cores
        idxw = np.tile(idx.reshape(-1, 16).T, (8, 1)).copy()
        ldstT = np.ascontiguousarray(ldst.reshape(-1, 128).T)
        valT = np.ascontiguousarray(val.reshape(-1, 128).T)
        degs = np.ascontiguousarray(
            deg[c * PADSHARD:(c + 1) * PADSHARD].reshape(NT, 128).T
        ).astype(np.float32)
        idx_all.append(idxw)
        ldst_all.append(ldstT)
        val_all.append(valT)
        deg_all.append(degs)
    return idx_all, ldst_all, val_all, deg_all


# --------------------------------------------------------- device program ---
def _build(cfg, timing=False, nzbias=False):
    import os
    from concourse import bacc, tile
    import concourse.mybir as mybir
    kdbg = bool(os.environ.get("KDBG"))

    f32 = mybir.dt.float32
    bf16 = mybir.dt.bfloat16
    i16 = mybir.dt.int16
    AOP = mybir.AluOpType
    ACT = mybir.ActivationFunctionType

    nc = bacc.Bacc("TRN2", target_bir_lowering=False, debug=False,
                   num_devices=1 if timing else cfg.M,
                   dynamic_dma_scratch_size=16384,
                   num_swdge_queues=cfg.NQUEUES)

    NPAD, QROWS, NT, KSEG, CQ, SLABC, NSLAB = (
        cfg.NPAD, cfg.QROWS, cfg.NT, cfg.KSEG, cfg.CQ, cfg.SLABC, cfg.NSLAB)
    CHUNKS, HID, NCLS, IN_DIM = cfg.CHUNKS, cfg.HID, cfg.NCLS, cfg.IN_DIM
    LQ16 = CQ * 128 // 16              # idx columns per quarter
    SLAB16 = SLABC * 128 // 16         # idx columns per slab

    # -------- I/O
    XG = nc.dram_tensor("xg", [NPAD, IN_DIM], bf16, kind="ExternalInput")
    IDX = nc.dram_tensor("idx", [128, 4 * LQ16], i16, kind="ExternalInput")
    LDST = nc.dram_tensor("ldst", [128, CHUNKS], f32, kind="ExternalInput")
    VAL = nc.dram_tensor("val", [128, CHUNKS], f32, kind="ExternalInput")
    DEG = nc.dram_tensor("deg", [128, NT], f32, kind="ExternalInput")
    W1 = nc.dram_tensor("w1", [IN_DIM, HID], bf16, kind="ExternalInput")
    W2C = nc.dram_tensor("w2c", [HID, NCLS], bf16, kind="ExternalInput")
    B1R = nc.dram_tensor("b1r", [128, HID], f32, kind="ExternalInput")
    BCOMBR = nc.dram_tensor("bcombr", [128, NCLS], f32, kind="ExternalInput")
    BCR = nc.dram_tensor("bcr", [128, NCLS], f32, kind="ExternalInput")
    IOTA = nc.dram_tensor("iota", [128, 128], bf16, kind="ExternalInput")
    OUT = nc.dram_tensor("out", [cfg.PADSHARD, NCLS], f32, kind="ExternalOutput")
    HDBG = (nc.dram_tensor("hdbg", [cfg.PADSHARD, HID], bf16,
                           kind="ExternalOutput") if kdbg else None)
    LDBG = (nc.dram_tensor("ldbg", [128, NT * NCLS], f32,
                           kind="ExternalOutput") if kdbg else None)
    SMDBG = (nc.dram_tensor("smdbg", [128, NT], f32,
                            kind="ExternalOutput") if kdbg else None)

    # -------- internal DRAM
    HS = nc.dram_tensor("hshard", [cfg.PADSHARD, 128], bf16)    # cols 64+: junk
    HF = nc.dram_tensor("hfull", [NPAD, 128], bf16, addr_space="Shared")

    with tile.TileContext(nc) as tc, ExitStack() as top:
        cpool = top.enter_context(tc.tile_pool(name="consts", bufs=1))
        w1s = cpool.tile([IN_DIM, HID], bf16)
        nc.sync.dma_start(out=w1s, in_=W1[:, :])
        w2cs = cpool.tile([HID, NCLS], bf16)
        nc.sync.dma_start(out=w2cs, in_=W2C[:, :])
        b1rs = cpool.tile([128, HID], f32)
        nc.sync.dma_start(out=b1rs, in_=B1R[:, :])
        bcombs = cpool.tile([128, NCLS], f32)
        nc.sync.dma_start(out=bcombs, in_=BCOMBR[:, :])
        bcrs = cpool.tile([128, NCLS], f32)
        nc.sync.dma_start(out=bcrs, in_=BCR[:, :])
        iot = cpool.tile([128, 128], bf16)
        nc.sync.dma_start(out=iot, in_=IOTA[:, :])
        degs = cpool.tile([128, NT], f32)
        nc.sync.dma_start(out=degs, in_=DEG[:, :])

        edg = top.enter_context(tc.tile_pool(name="edg", bufs=1))
        ldsts = edg.tile([128, CHUNKS], f32)
        nc.sync.dma_start(out=ldsts, in_=LDST[:, :])
        vals = edg.tile([128, CHUNKS], f32)
        nc.sync.dma_start(out=vals, in_=VAL[:, :])

        # idx tiles for BOTH layers, loaded up front
        idxp = top.enter_context(tc.tile_pool(name="idxp", bufs=1))
        its = []
        for li in range(2):
            row = []
            for q in range(4):
                it = idxp.tile([128, LQ16], i16, tag=f"idx{li}_{q}",
                               name=f"idx{li}_{q}")
                nc.sync.dma_start(out=it, in_=IDX[:, q * LQ16:(q + 1) * LQ16])
                row.append(it)
            its.append(row)

        # ============ spmm layer runner: per-tile single psum group across
        # all 4 quarters, accumulating transposed (psT = Xg^T V); epilogue
        # split into a per-tile part (cast) and a batched per-GE-tiles part
        # (weight matmuls etc) to keep the PE stream free of cross-engine
        # round trips.
        def spmm_layer(tab, width, lits, epi_tile, epi_group, pools):
            msg, vp, psb = pools
            GEB = cfg.GE
            slabs = [[None] * NSLAB for _ in range(4)]

            def ensure_slab(q, s):
                if slabs[q][s] is None:
                    mt = msg.tile([128, SLABC, 128], bf16)
                    nc.gpsimd.dma_gather(
                        mt, tab[q * QROWS:(q + 1) * QROWS, :],
                        lits[q][:, s * SLAB16:(s + 1) * SLAB16],
                        num_idxs=SLABC * 128, num_idxs_reg=SLABC * 128,
                        elem_size=128, elem_step=128,
                        single_packet=cfg.SINGLE_PACKET,
                        queue_num=(q * NSLAB + s) % cfg.NQUEUES)
                    slabs[q][s] = mt
                return slabs[q][s]

            for t in range(NT):
                ps = psb.tile([width, 128], f32)
                for q in range(4):
                    j0 = t * KSEG
                    vt = vp.tile([128, KSEG, 128], bf16)
                    for k in range(KSEG):
                        gj = q * CQ + j0 + k             # global chunk
                        veng = (nc.gpsimd
                                if q * KSEG + k >= 16 - cfg.POOLV
                                else nc.vector)
                        veng.tensor_scalar(
                            vt[:, k, :], iot, ldsts[:, gj:gj + 1],
                            vals[:, gj:gj + 1], AOP.is_equal, AOP.mult)
                    for k in range(KSEG):
                        j = j0 + k                       # chunk in quarter
                        mt = ensure_slab(q, j // SLABC)
                        nc.tensor.matmul(ps, lhsT=mt[:, j % SLABC, 0:width],
                                         rhs=vt[:, k, :],
                                         start=(q == 0 and k == 0),
                                         stop=(q == 3 and k == KSEG - 1))
                epi_tile(t, ps)
                if t % GEB == GEB - 1:
                    epi_group(t - GEB + 1, GEB)

        # ================= layer 1: h = relu((A x)@W1 + deg*b1^T), store bf16
        with tc.tile_pool(name="msg", bufs=cfg.MSGBUFS) as msg, \
             tc.tile_pool(name="vp", bufs=8) as vp, \
             tc.tile_pool(name="psb", bufs=3, space="PSUM") as psb, \
             tc.tile_pool(name="tc1", bufs=cfg.GE + 2) as tp1, \
             tc.tile_pool(name="tc2", bufs=3) as tp2, \
             tc.tile_pool(name="pse", bufs=3, space="PSUM") as pse:
            pss1 = {}

            def epi1_tile(t, ps):
                pss = tp1.tile([IN_DIM, 128], bf16, tag="pss", name="pss")
                nc.scalar.activation(pss, ps, ACT.Copy)
                pss1[t] = pss

            def epi1_group(t0, n):
                for t in range(t0, t0 + n):
                    ph = pse.tile([128, HID], f32)
                    nc.tensor.matmul(ph, lhsT=pss1.pop(t), rhs=w1s,
                                     start=True, stop=True)
                    ht = tp2.tile([128, HID], bf16, tag="ht")
                    if nzbias:
                        tb = tp2.tile([128, HID], f32, tag="tb")
                        nc.vector.tensor_scalar(tb, b1rs, degs[:, t:t + 1],
                                                None, AOP.mult)
                        hsum = tp2.tile([128, HID], f32, tag="hsum")
                        nc.vector.tensor_tensor(hsum, ph, tb, AOP.add)
                        nc.scalar.activation(ht, hsum, ACT.Relu)
                    else:
                        nc.scalar.activation(ht, ph, ACT.Relu)
                    nc.sync.dma_start(out=HS[t * 128:(t + 1) * 128, 0:HID],
                                      in_=ht)

            spmm_layer(XG, IN_DIM, its[0], epi1_tile, epi1_group,
                       (msg, vp, psb))
            if kdbg:
                nc.sync.dma_start(out=HDBG[:, :], in_=HS[:, 0:HID])
            if not timing:
                nc.gpsimd.collective_compute(
                    "AllGather", mybir.AluOpType.bypass,
                    replica_groups=[list(range(cfg.M))],
                    ins=[HS[:, :]], outs=[HF[:, :]])

        # ================= layer 2 + fused classifier/log_softmax
        with tc.tile_pool(name="msg2", bufs=cfg.MSGBUFS) as msg2, \
             tc.tile_pool(name="vp2", bufs=8) as vp2, \
             tc.tile_pool(name="psb2", bufs=3, space="PSUM") as psb2, \
             tc.tile_pool(name="te1", bufs=cfg.GE + 2) as te1, \
             tc.tile_pool(name="te2", bufs=3) as te2, \
             tc.tile_pool(name="te3", bufs=2) as te3, \
             tc.tile_pool(name="psf", bufs=3, space="PSUM") as psf:
            G = cfg.LNG
            assert NT % G == 0 and G % cfg.GE == 0
            pss2 = {}
            state = {}

            def epi2_tile(t, ps):
                pss = te1.tile([HID, 128], bf16, tag="pss", name="pss")
                nc.scalar.activation(pss, ps, ACT.Copy)
                pss2[t] = pss

            def epi2_group(t0, n):
                for t in range(t0, t0 + n):
                    g, i = t // G, t % G
                    if i == 0:
                        state["lgg"] = te3.tile([128, G, NCLS], f32,
                                                tag="lgg", name="lgg")
                        state["negg"] = te3.tile([128, G], f32,
                                                 tag="negg", name="negg")
                        state["smg"] = te3.tile([128, G], f32,
                                                tag="smg", name="smg")
                    lgg, negg, smg = state["lgg"], state["negg"], state["smg"]
                    psl = psf.tile([128, NCLS], f32)
                    nc.tensor.matmul(psl, lhsT=pss2.pop(t), rhs=w2cs,
                                     start=True, stop=True)
                    if nzbias:
                        tb = te2.tile([128, NCLS], f32, tag="tb")
                        nc.vector.tensor_scalar(tb, bcombs, degs[:, t:t + 1],
                                                None, AOP.mult)
                        lg0 = te2.tile([128, NCLS], f32, tag="lg0")
                        nc.vector.tensor_tensor(lg0, psl, tb, AOP.add)
                        nc.gpsimd.tensor_tensor(lgg[:, i, :], lg0, bcrs,
                                                AOP.add)
                    else:
                        nc.scalar.activation(lgg[:, i, :], psl, ACT.Copy)
                    if i == G - 1:
                        if kdbg:
                            nc.sync.dma_start(
                                out=LDBG[:, g * G * NCLS:(g + 1) * G * NCLS],
                                in_=lgg.rearrange("p a b -> p (a b)"))
                            nc.sync.dma_start(out=SMDBG[:, g * G:(g + 1) * G],
                                              in_=smg)
                        nc.vector.tensor_reduce(negg[:, :], lgg,
                                                mybir.AxisListType.X, AOP.max,
                                                negate=True)
                        for ii in range(G):
                            et = te2.tile([128, NCLS], f32, tag="et")
                            nc.scalar.activation(et, lgg[:, ii, :], ACT.Exp,
                                                 bias=negg[:, ii:ii + 1],
                                                 accum_out=smg[:, ii:ii + 1])
                        lng = te2.tile([128, G], f32, tag="lng")
                        nc.scalar.activation(lng, smg, ACT.Ln)
                        shg = te2.tile([128, G], f32, tag="shg")
                        nc.vector.tensor_tensor(shg, negg, lng, AOP.subtract)
                        for ii in range(G):
                            tt = g * G + ii
                            ot = te2.tile([128, NCLS], f32, tag="ot")
                            nc.vector.tensor_scalar(ot, lgg[:, ii, :],
                                                    shg[:, ii:ii + 1], None,
                                                    AOP.add)
                            nc.sync.dma_start(
                                out=OUT[tt * 128:(tt + 1) * 128, :], in_=ot)

            spmm_layer(HF, HID, its[1], epi2_tile, epi2_group,
                       (msg2, vp2, psb2))

    nc.compile()
    return nc


_NC_CACHE = {}


def _get_nc(cfg):
    key = (cfg.NT, cfg.KSEG, cfg.SLABC, cfg.NZBIAS)
    if key not in _NC_CACHE:
        _NC_CACHE[key] = _build(cfg, nzbias=cfg.NZBIAS)
    return _NC_CACHE[key]


# ------------------------------------------------------------------ main ---
def kernel(x, edge_row, edge_col, edge_val, W1, b1, W2, b2, Wc, bc,
           _run_kwargs=None):
    from concourse.bass_utils import run_bass_kernel_spmd

    cfg = CFG
    x = np.asarray(x, dtype=np.float32)
    edge_row = np.asarray(edge_row, dtype=np.int64)
    edge_col = np.asarray(edge_col, dtype=np.int64)
    edge_val = np.asarray(edge_val, dtype=np.float32)
    W1 = np.asarray(W1, dtype=np.float32)
    W2 = np.asarray(W2, dtype=np.float32)
    Wc = np.asarray(Wc, dtype=np.float32)
    b1 = np.asarray(b1, dtype=np.float32)
    b2 = np.asarray(b2, dtype=np.float32)
    bc = np.asarray(bc, dtype=np.float32)

    cfg.NZBIAS = bool(np.any(b1) or np.any(b2) or np.any(bc))
    slot_of = _assign_slots(cfg, edge_row, edge_col)
    try:
        idx_all, ldst_all, val_all, deg_all = _plan(
            cfg, edge_row, edge_col, edge_val, slot_of)
    except ValueError:
        cfg.KSEG += 1
        idx_all, ldst_all, val_all, deg_all = _plan(
            cfg, edge_row, edge_col, edge_val, slot_of)

    xg = np.zeros((cfg.NPAD, cfg.IN_DIM), dtype=ml_dtypes.bfloat16)
    xg[slot_of] = x.astype(ml_dtypes.bfloat16)

    w1h = W1.astype(ml_dtypes.bfloat16)
    w2c = (W2 @ Wc).astype(ml_dtypes.bfloat16)
    bcomb = b2 @ Wc
    iota = np.tile(np.arange(128, dtype=np.float32), (128, 1)).astype(
        ml_dtypes.bfloat16)
    b1r = np.tile(b1, (128, 1)).astype(np.float32)
    bcombr = np.tile(bcomb, (128, 1)).astype(np.float32)
    bcr = np.tile(bc, (128, 1)).astype(np.float32)

    nc = _get_nc(cfg)
    in_maps = []
    for c in range(cfg.M):
        in_maps.append({
            "xg": xg, "idx": idx_all[c], "ldst": ldst_all[c],
            "val": val_all[c], "deg": deg_all[c], "w1": w1h, "w2c": w2c,
            "b1r": b1r, "bcombr": bcombr, "bcr": bcr, "iota": iota,
        })
    kw = dict(_run_kwargs or {})
    res = run_bass_kernel_spmd(nc, in_maps, core_ids=list(range(cfg.M)), **kw)
    shard = np.concatenate(
        [res.results[c]["out"] for c in range(cfg.M)], axis=0)  # [NPAD, NCLS]
    out = shard[slot_of]
    kernel.last_results = res
    return out.astype(np.float32)


# revision 27
# speedup vs baseline: 1.8172x; 1.0205x over previous
"""GCN node classifier (2x spmm + classifier + log_softmax) on 8 trn2 cores.

Strategy: destination-node 1D sharding with spmm linearity.
  spmm(A, x@W1 + b1) = (A x)@W1 + deg * b1^T      (deg = rowsum of A)
  spmm(A, h@W2 + b2)@Wc = (A h)@(W2 Wc) + deg * (b2 Wc)^T
so the gather tables are the RAW node features (x bf16 for layer 1,
relu-h bf16 for layer 2) — no dense pre-pass over all nodes, and the
layer weights are applied per dst tile after aggregation.

Each core owns 12,800 dst slots (100 tiles x 128 lanes). Host assigns
nodes to slots with a greedy 4-d balancer so that every (src-quarter,
dst-tile) edge bucket fits in KSEG=4 chunks of 128 edges (the int16
gather index forces 4 quarter views of the 102,400-row table). Per-edge
source rows are fetched with GPSIMD dma_gather (256B rows); the
segment-sum is a tensor-engine matmul against per-chunk scatter
matrices V[e, dst_lane] = edge_val[e] built on DVE with
(iota == ldst) * val, accumulated transposed (psT = Xg^T V) so the
per-tile epilogue can feed psT straight back as lhsT for the weight
matmul. log-softmax is fused per tile. Between layers the per-shard
relu-h table is AllGather'ed into a Shared DRAM tensor.
"""

import numpy as np
import ml_dtypes

from contextlib import ExitStack


# ---------------------------------------------------------------- config ---
class Cfg:
    M = 8                 # cores
    N_NODES = 100000
    N_EDGES = 1600000
    IN_DIM = 128
    HID = 64
    NCLS = 40
    NT = 100              # dst tiles per core (128 lanes each)
    KSEG = 4              # chunks (of 128 edges) per (quarter, tile) segment
    SLABC = 10            # chunks per gather slab
    SINGLE_PACKET = False  # multi-packet gathers (single-packet hangs >~1K idxs)
    NQUEUES = 4           # spread gathers over all 4 SWDGE queues
    MSGBUFS = 28
    IDXBUFS = 1
    GE = 5                # tiles per epilogue-matmul batch
    POOLV = 1             # of 16 chunks/tile, how many V-builds go to Pool
    LNG = 5               # tiles per deferred-Ln group
    NZBIAS = False        # set per-input: any of b1/b2/bc nonzero

    @property
    def PADSHARD(self):
        return self.NT * 128

    @property
    def NPAD(self):
        return self.PADSHARD * self.M

    @property
    def QROWS(self):
        return self.NPAD // 4

    @property
    def SEG(self):
        return self.KSEG * 128

    @property
    def CQ(self):
        return self.NT * self.KSEG          # chunks per quarter

    @property
    def NSLAB(self):
        assert self.CQ % self.SLABC == 0
        return self.CQ // self.SLABC        # gather slabs per quarter

    @property
    def CHUNKS(self):
        return 4 * self.CQ


CFG = Cfg()


# ------------------------------------------------------------- host plan ---
def _assign_slots(cfg, edge_row, edge_col):
    """Assign nodes to table slots so every (src-quarter, dst-tile) edge
    bucket holds <= KSEG*128 edges. Returns slot_of[node] -> [0, NPAD).

    Nodes are first split into 4 fixed quarter groups (so each node's
    src-quarter is pinned), then greedily packed into the 2*NT tiles of
    their own quarter balancing the 4-vector of per-src-quarter in-edge
    counts.
    """
    N, NPAD, QROWS, NT, M = cfg.N_NODES, cfg.NPAD, cfg.QROWS, cfg.NT, cfg.M
    TPQ = QROWS // 128                       # tiles per quarter (2 cores)
    rng = np.random.default_rng(12345)
    order = rng.permutation(N)
    qgrp = np.empty(N, dtype=np.int64)       # node -> quarter group
    npq = N // 4
    for q in range(4):
        qgrp[order[q * npq:(q + 1) * npq]] = q
    qgrp[order[4 * npq:]] = 3

    # per-node in-edge count by source quarter
    cnt = np.zeros((N, 4), dtype=np.int64)
    np.add.at(cnt, (edge_row, qgrp[edge_col]), 1)

    slot_of = np.empty(N, dtype=np.int64)
    for q in range(4):
        nodes = np.where(qgrp == q)[0]
        c = cnt[nodes].astype(np.float32)            # [nq, 4]
        tot = c.sum(1)
        o = np.argsort(-tot, kind="stable")
        nodes, c = nodes[o], c[o]
        loads = np.zeros((TPQ, 4), dtype=np.float32)
        fill = np.zeros(TPQ, dtype=np.int64)
        pos = np.empty(nodes.size, dtype=np.int64)
        for i in range(nodes.size):
            cand = np.max(loads + c[i], axis=1) + (fill >= 128) * 1e9
            b = int(np.argmin(cand))
            loads[b] += c[i]
            pos[i] = b * 128 + fill[b]
            fill[b] += 1
        slot_of[nodes] = q * QROWS + pos
    return slot_of


def _plan(cfg, edge_row, edge_col, edge_val, slot_of):
    """Bucket/sort/pad edges per core. Returns per-core arrays:
    idx16 [128, 4*CQ*128/16] int16, ldstT/valT [128, CHUNKS] bf16,
    plus degs [128, NT] f32 per core.
    """
    M, NT, KSEG, SEG, CQ, QROWS = cfg.M, cfg.NT, cfg.KSEG, cfg.SEG, cfg.CQ, cfg.QROWS
    PADSHARD = cfg.PADSHARD

    src_slot = slot_of[edge_col]
    dst_slot = slot_of[edge_row]
    q_of = src_slot // QROWS
    i_of = src_slot % QROWS
    core_of = dst_slot // PADSHARD
    dloc = dst_slot % PADSHARD
    t_of = dloc // 128
    l_of = dloc % 128

    deg = np.zeros(cfg.NPAD, dtype=np.float64)
    np.add.at(deg, dst_slot, edge_val.astype(np.float64))

    L = 4 * CQ * 128
    idx_all, ldst_all, val_all, deg_all = [], [], [], []
    for c in range(M):
        sel = core_of == c
        segid = q_of[sel] * NT + t_of[sel]
        order = np.argsort(segid, kind="stable")
        sid = segid[order]
        idx_s = i_of[sel][order]
        l_s = l_of[sel][order]
        v_s = edge_val[sel][order]

        counts = np.bincount(sid, minlength=4 * NT)
        if counts.max() > SEG:
            raise ValueError(f"segment overflow: {counts.max()} > {SEG}")
        starts = np.arange(4 * NT) * SEG
        pos = starts[sid] + (np.arange(sid.size)
                             - np.concatenate(([0], np.cumsum(counts)))[sid])

        idx = np.zeros(L, dtype=np.int16)
        ldst = np.zeros(L, dtype=np.float32)
        val = np.zeros(L, dtype=np.float32)
        idx[pos] = idx_s.astype(np.int16)
        ldst[pos] = l_s.astype(np.float32)
        val[pos] = v_s.astype(np.float32)

        # wrap indices: idx i -> [i%16, i//16], replicated on all 8 q7 cores
        idxw = np.tile(idx.reshape(-1, 16).T, (8, 1)).copy()
        ldstT = np.ascontiguousarray(ldst.reshape(-1, 128).T)
        valT = np.ascontiguousarray(val.reshape(-1, 128).T)
        degs = np.ascontiguousarray(
            deg[c * PADSHARD:(c + 1) * PADSHARD].reshape(NT, 128).T
        ).astype(np.float32)
        idx_all.append(idxw)
        ldst_all.append(ldstT)
        val_all.append(valT)
        deg_all.append(degs)
    return idx_all, ldst_all, val_all, deg_all


# --------------------------------------------------------- device program ---
def _build(cfg, timing=False, nzbias=False):
    import os
    from concourse import bacc, tile
    import concourse.mybir as mybir
    kdbg = bool(os.environ.get("KDBG"))

    f32 = mybir.dt.float32
    bf16 = mybir.dt.bfloat16
    i16 = mybir.dt.int16
    AOP = mybir.AluOpType
    ACT = mybir.ActivationFunctionType

    nc = bacc.Bacc("TRN2", target_bir_lowering=False, debug=False,
                   num_devices=1 if timing else cfg.M,
                   dynamic_dma_scratch_size=16384,
                   num_swdge_queues=cfg.NQUEUES)

    NPAD, QROWS, NT, KSEG, CQ, SLABC, NSLAB = (
        cfg.NPAD, cfg.QROWS, cfg.NT, cfg.KSEG, cfg.CQ, cfg.SLABC, cfg.NSLAB)
    CHUNKS, HID, NCLS, IN_DIM = cfg.CHUNKS, cfg.HID, cfg.NCLS, cfg.IN_DIM
    LQ16 = CQ * 128 // 16              # idx columns per quarter
    SLAB16 = SLABC * 128 // 16         # idx columns per slab

    # -------- I/O
    XG = nc.dram_tensor("xg", [NPAD, IN_DIM], bf16, kind="ExternalInput")
    IDX = nc.dram_tensor("idx", [128, 4 * LQ16], i16, kind="ExternalInput")
    LDST = nc.dram_tensor("ldst", [128, CHUNKS], f32, kind="ExternalInput")
    VAL = nc.dram_tensor("val", [128, CHUNKS], f32, kind="ExternalInput")
    DEG = nc.dram_tensor("deg", [128, NT], f32, kind="ExternalInput")
    W1 = nc.dram_tensor("w1", [IN_DIM, HID], bf16, kind="ExternalInput")
    W2C = nc.dram_tensor("w2c", [HID, NCLS], bf16, kind="ExternalInput")
    B1R = nc.dram_tensor("b1r", [128, HID], f32, kind="ExternalInput")
    BCOMBR = nc.dram_tensor("bcombr", [128, NCLS], f32, kind="ExternalInput")
    BCR = nc.dram_tensor("bcr", [128, NCLS], f32, kind="ExternalInput")
    IOTA = nc.dram_tensor("iota", [128, 128], bf16, kind="ExternalInput")
    f16 = mybir.dt.float16
    OUT = nc.dram_tensor("out", [cfg.PADSHARD, NCLS], f16, kind="ExternalOutput")
    HDBG = (nc.dram_tensor("hdbg", [cfg.PADSHARD, HID], bf16,
                           kind="ExternalOutput") if kdbg else None)
    LDBG = (nc.dram_tensor("ldbg", [128, NT * NCLS], f32,
                           kind="ExternalOutput") if kdbg else None)
    SMDBG = (nc.dram_tensor("smdbg", [128, NT], f32,
                            kind="ExternalOutput") if kdbg else None)

    # -------- internal DRAM
    HS = nc.dram_tensor("hshard", [cfg.PADSHARD, 128], bf16)    # cols 64+: junk
    HF = nc.dram_tensor("hfull", [NPAD, 128], bf16, addr_space="Shared")

    with tile.TileContext(nc) as tc, ExitStack() as top:
        # idx quarter 0 + V-build operands first: the first gathers and
        # V-builds depend only on these, so they issue before the consts.
        idxp = top.enter_context(tc.tile_pool(name="idxp", bufs=1))
        its = [[None] * 4 for _ in range(2)]
        its[0][0] = idxp.tile([128, LQ16], i16, tag="idx0_0", name="idx0_0")
        nc.sync.dma_start(out=its[0][0], in_=IDX[:, 0:LQ16])

        cpool = top.enter_context(tc.tile_pool(name="consts", bufs=1))
        iot = cpool.tile([128, 128], bf16)
        nc.sync.dma_start(out=iot, in_=IOTA[:, :])

        edg = top.enter_context(tc.tile_pool(name="edg", bufs=1))
        ldsts = edg.tile([128, CHUNKS], f32)
        nc.sync.dma_start(out=ldsts, in_=LDST[:, :])
        vals = edg.tile([128, CHUNKS], f32)
        nc.sync.dma_start(out=vals, in_=VAL[:, :])

        for q in range(1, 4):
            its[0][q] = idxp.tile([128, LQ16], i16, tag=f"idx0_{q}",
                                  name=f"idx0_{q}")
            nc.sync.dma_start(out=its[0][q],
                              in_=IDX[:, q * LQ16:(q + 1) * LQ16])

        w1s = cpool.tile([IN_DIM, HID], bf16)
        nc.sync.dma_start(out=w1s, in_=W1[:, :])
        w2cs = cpool.tile([HID, NCLS], bf16)
        nc.sync.dma_start(out=w2cs, in_=W2C[:, :])
        b1rs = cpool.tile([128, HID], f32)
        nc.sync.dma_start(out=b1rs, in_=B1R[:, :])
        bcombs = cpool.tile([128, NCLS], f32)
        nc.sync.dma_start(out=bcombs, in_=BCOMBR[:, :])
        bcrs = cpool.tile([128, NCLS], f32)
        nc.sync.dma_start(out=bcrs, in_=BCR[:, :])
        degs = cpool.tile([128, NT], f32)
        nc.sync.dma_start(out=degs, in_=DEG[:, :])

        for q in range(4):
            its[1][q] = idxp.tile([128, LQ16], i16, tag=f"idx1_{q}",
                                  name=f"idx1_{q}")
            nc.sync.dma_start(out=its[1][q],
                              in_=IDX[:, q * LQ16:(q + 1) * LQ16])

        # ============ spmm layer runner: per-tile single psum group across
        # all 4 quarters, accumulating transposed (psT = Xg^T V); epilogue
        # split into a per-tile part (cast) and a batched per-GE-tiles part
        # (weight matmuls etc) to keep the PE stream free of cross-engine
        # round trips.
        def spmm_layer(tab, width, lits, epi_tile, epi_group, pools):
            msg, vp, psb = pools
            GEB = cfg.GE
            slabs = [[None] * NSLAB for _ in range(4)]

            def ensure_slab(q, s):
                if slabs[q][s] is None:
                    mt = msg.tile([128, SLABC, 128], bf16)
                    nc.gpsimd.dma_gather(
                        mt, tab[q * QROWS:(q + 1) * QROWS, :],
                        lits[q][:, s * SLAB16:(s + 1) * SLAB16],
                        num_idxs=SLABC * 128, num_idxs_reg=SLABC * 128,
                        elem_size=128, elem_step=128,
                        single_packet=cfg.SINGLE_PACKET,
                        queue_num=(q * NSLAB + s) % cfg.NQUEUES)
                    slabs[q][s] = mt
                return slabs[q][s]

            for t in range(NT):
                ps = psb.tile([width, 128], f32)
                for q in range(4):
                    j0 = t * KSEG
                    vt = vp.tile([128, KSEG, 128], bf16)
                    for k in range(KSEG):
                        gj = q * CQ + j0 + k             # global chunk
                        veng = (nc.gpsimd
                                if q * KSEG + k >= 16 - cfg.POOLV
                                else nc.vector)
                        veng.tensor_scalar(
                            vt[:, k, :], iot, ldsts[:, gj:gj + 1],
                            vals[:, gj:gj + 1], AOP.is_equal, AOP.mult)
                    for k in range(KSEG):
                        j = j0 + k                       # chunk in quarter
                        mt = ensure_slab(q, j // SLABC)
                        nc.tensor.matmul(ps, lhsT=mt[:, j % SLABC, 0:width],
                                         rhs=vt[:, k, :],
                                         start=(q == 0 and k == 0),
                                         stop=(q == 3 and k == KSEG - 1))
                epi_tile(t, ps)
                if t % GEB == GEB - 1:
                    epi_group(t - GEB + 1, GEB)

        # ================= layer 1: h = relu((A x)@W1 + deg*b1^T), store bf16
        with tc.tile_pool(name="msg", bufs=cfg.MSGBUFS) as msg, \
             tc.tile_pool(name="vp", bufs=8) as vp, \
             tc.tile_pool(name="psb", bufs=3, space="PSUM") as psb, \
             tc.tile_pool(name="tc1", bufs=cfg.GE + 2) as tp1, \
             tc.tile_pool(name="tc2", bufs=3) as tp2, \
             tc.tile_pool(name="pse", bufs=3, space="PSUM") as pse:
            pss1 = {}

            def epi1_tile(t, ps):
                pss = tp1.tile([IN_DIM, 128], bf16, tag="pss", name="pss")
                nc.scalar.activation(pss, ps, ACT.Copy)
                pss1[t] = pss

            def epi1_group(t0, n):
                for t in range(t0, t0 + n):
                    ph = pse.tile([128, HID], f32)
                    nc.tensor.matmul(ph, lhsT=pss1.pop(t), rhs=w1s,
                                     start=True, stop=True)
                    ht = tp2.tile([128, HID], bf16, tag="ht")
                    if nzbias:
                        tb = tp2.tile([128, HID], f32, tag="tb")
                        nc.vector.tensor_scalar(tb, b1rs, degs[:, t:t + 1],
                                                None, AOP.mult)
                        hsum = tp2.tile([128, HID], f32, tag="hsum")
                        nc.vector.tensor_tensor(hsum, ph, tb, AOP.add)
                        nc.scalar.activation(ht, hsum, ACT.Relu)
                    else:
                        nc.scalar.activation(ht, ph, ACT.Relu)
                    nc.sync.dma_start(out=HS[t * 128:(t + 1) * 128, 0:HID],
                                      in_=ht)

            spmm_layer(XG, IN_DIM, its[0], epi1_tile, epi1_group,
                       (msg, vp, psb))
            if kdbg:
                nc.sync.dma_start(out=HDBG[:, :], in_=HS[:, 0:HID])
            if not timing:
                nc.gpsimd.collective_compute(
                    "AllGather", mybir.AluOpType.bypass,
                    replica_groups=[list(range(cfg.M))],
                    ins=[HS[:, :]], outs=[HF[:, :]])

        # ================= layer 2 + fused classifier/log_softmax
        with tc.tile_pool(name="msg2", bufs=cfg.MSGBUFS) as msg2, \
             tc.tile_pool(name="vp2", bufs=8) as vp2, \
             tc.tile_pool(name="psb2", bufs=3, space="PSUM") as psb2, \
             tc.tile_pool(name="te1", bufs=cfg.GE + 2) as te1, \
             tc.tile_pool(name="te2", bufs=3) as te2, \
             tc.tile_pool(name="te3", bufs=2) as te3, \
             tc.tile_pool(name="psf", bufs=3, space="PSUM") as psf:
            G = cfg.LNG
            assert NT % G == 0 and G % cfg.GE == 0
            pss2 = {}
            state = {}

            def epi2_tile(t, ps):
                pss = te1.tile([HID, 128], bf16, tag="pss", name="pss")
                nc.scalar.activation(pss, ps, ACT.Copy)
                pss2[t] = pss

            def epi2_group(t0, n):
                for t in range(t0, t0 + n):
                    g, i = t // G, t % G
                    if i == 0:
                        state["lgg"] = te3.tile([128, G, NCLS], f32,
                                                tag="lgg", name="lgg")
                        state["negg"] = te3.tile([128, G], f32,
                                                 tag="negg", name="negg")
                        state["smg"] = te3.tile([128, G], f32,
                                                tag="smg", name="smg")
                    lgg, negg, smg = state["lgg"], state["negg"], state["smg"]
                    psl = psf.tile([128, NCLS], f32)
                    nc.tensor.matmul(psl, lhsT=pss2.pop(t), rhs=w2cs,
                                     start=True, stop=True)
                    if nzbias:
                        tb = te2.tile([128, NCLS], f32, tag="tb")
                        nc.vector.tensor_scalar(tb, bcombs, degs[:, t:t + 1],
                                                None, AOP.mult)
                        lg0 = te2.tile([128, NCLS], f32, tag="lg0")
                        nc.vector.tensor_tensor(lg0, psl, tb, AOP.add)
                        nc.gpsimd.tensor_tensor(lgg[:, i, :], lg0, bcrs,
                                                AOP.add)
                    else:
                        nc.scalar.activation(lgg[:, i, :], psl, ACT.Copy)
                    if i == G - 1:
                        if kdbg:
                            nc.sync.dma_start(
                                out=LDBG[:, g * G * NCLS:(g + 1) * G * NCLS],
                                in_=lgg.rearrange("p a b -> p (a b)"))
                            nc.sync.dma_start(out=SMDBG[:, g * G:(g + 1) * G],
                                              in_=smg)
                        nc.vector.tensor_reduce(negg[:, :], lgg,
                                                mybir.AxisListType.X, AOP.max,
                                                negate=True)
                        for ii in range(G):
                            et = te2.tile([128, NCLS], f32, tag="et")
                            nc.scalar.activation(et, lgg[:, ii, :], ACT.Exp,
                                                 bias=negg[:, ii:ii + 1],
                                                 accum_out=smg[:, ii:ii + 1])
                        lng = te2.tile([128, G], f32, tag="lng")
                        nc.scalar.activation(lng, smg, ACT.Ln)
                        shg = te2.tile([128, G], f32, tag="shg")
                        nc.vector.tensor_tensor(shg, negg, lng, AOP.subtract)
                        for ii in range(G):
                            tt = g * G + ii
                            ot = te2.tile([128, NCLS], f16, tag="ot")
                            nc.vector.tensor_scalar(ot, lgg[:, ii, :],
                                                    shg[:, ii:ii + 1], None,
                                                    AOP.add)
                            nc.sync.dma_start(
                                out=OUT[tt * 128:(tt + 1) * 128, :], in_=ot)

            spmm_layer(HF, HID, its[1], epi2_tile, epi2_group,
                       (msg2, vp2, psb2))

    nc.compile()
    return nc


_NC_CACHE = {}


def _get_nc(cfg):
    key = (cfg.NT, cfg.KSEG, cfg.SLABC, cfg.NZBIAS)
    if key not in _NC_CACHE:
        _NC_CACHE[key] = _build(cfg, nzbias=cfg.NZBIAS)
    return _NC_CACHE[key]


# ------------------------------------------------------------------ main ---
def kernel(x, edge_row, edge_col, edge_val, W1, b1, W2, b2, Wc, bc,
           _run_kwargs=None):
    from concourse.bass_utils import run_bass_kernel_spmd

    cfg = CFG
    x = np.asarray(x, dtype=np.float32)
    edge_row = np.asarray(edge_row, dtype=np.int64)
    edge_col = np.asarray(edge_col, dtype=np.int64)
    edge_val = np.asarray(edge_val, dtype=np.float32)
    W1 = np.asarray(W1, dtype=np.float32)
    W2 = np.asarray(W2, dtype=np.float32)
    Wc = np.asarray(Wc, dtype=np.float32)
    b1 = np.asarray(b1, dtype=np.float32)
    b2 = np.asarray(b2, dtype=np.float32)
    bc = np.asarray(bc, dtype=np.float32)

    cfg.NZBIAS = bool(np.any(b1) or np.any(b2) or np.any(bc))
    slot_of = _assign_slots(cfg, edge_row, edge_col)
    try:
        idx_all, ldst_all, val_all, deg_all = _plan(
            cfg, edge_row, edge_col, edge_val, slot_of)
    except ValueError:
        cfg.KSEG += 1
        idx_all, ldst_all, val_all, deg_all = _plan(
            cfg, edge_row, edge_col, edge_val, slot_of)

    xg = np.zeros((cfg.NPAD, cfg.IN_DIM), dtype=ml_dtypes.bfloat16)
    xg[slot_of] = x.astype(ml_dtypes.bfloat16)

    w1h = W1.astype(ml_dtypes.bfloat16)
    w2c = (W2 @ Wc).astype(ml_dtypes.bfloat16)
    bcomb = b2 @ Wc
    iota = np.tile(np.arange(128, dtype=np.float32), (128, 1)).astype(
        ml_dtypes.bfloat16)
    b1r = np.tile(b1, (128, 1)).astype(np.float32)
    bcombr = np.tile(bcomb, (128, 1)).astype(np.float32)
    bcr = np.tile(bc, (128, 1)).astype(np.float32)

    nc = _get_nc(cfg)
    in_maps = []
    for c in range(cfg.M):
        in_maps.append({
            "xg": xg, "idx": idx_all[c], "ldst": ldst_all[c],
            "val": val_all[c], "deg": deg_all[c], "w1": w1h, "w2c": w2c,
            "b1r": b1r, "bcombr": bcombr, "bcr": bcr, "iota": iota,
        })
    kw = dict(_run_kwargs or {})
    res = run_bass_kernel_spmd(nc, in_maps, core_ids=list(range(cfg.M)), **kw)
    shard = np.concatenate(
        [res.results[c]["out"] for c in range(cfg.M)], axis=0)  # [NPAD, NCLS]
    out = shard[slot_of]
    kernel.last_results = res
    return out.astype(np.float32)
